# revision 9
# baseline (speedup 1.0000x reference)
"""Trainium2 Bass kernel for nn_Encoder_78795470012907.

Encoder layer: per-head Q/K/V projections, scores = QK^T/sqrt(dk),
double softmax (over batch axis, then over key axis), Z = pV, concat
heads, output projection. S=2048, B=4, D=512, H=8, dk=dv=64.

Sharding: head-parallel over 8 cores (core h owns head h) for the
attention; AllToAll re-shards by token for the output projection, so
each core emits a disjoint 1024-token slice of the output (host just
concatenates).

Layout notes (per core):
 - tokens are b-major: tok = b*2048 + s.
 - X is fed pre-transposed from host as XT [D, NTOK] (pure layout prep).
 - projections produce Q^T/K^T [dk, tok] stacked in b-pairs so the
   scores matmuls row-pack two batches into the 128x128 PE array.
 - scores are computed transposed ([t, s] tiles); the softmax over the
   key axis t rides the Z matmul via a ones-column appended to V
   (row 64 of the Z psum accumulates sum_t exp).
 - softmax over b: e=exp(s/8) -> D=sum_b e -> r=exp(-ln D) -> p1=e*r.
   ln+exp coexist in one ACT table set (no table thrashing);
   Reciprocal would force a table switch per tile.
"""

from contextlib import ExitStack

import numpy as np

import concourse.bass as bass
import concourse.tile as tile
from concourse import bacc, mybir
from concourse.bass_utils import run_bass_kernel_spmd

S, B, D = 2048, 4, 512
H, DK, DV = 8, 64, 64
N_CORES = 8
NTOK = S * B          # 8192 tokens, b-major
TOKC = NTOK // N_CORES  # 1024 tokens per core for the output slice
SC = 512              # s-chunk (columns of a scores^T tile)
TC = 128              # t-chunk (partitions of a scores^T tile)
N_SC = S // SC        # 4
N_TC = S // TC        # 16

F32 = mybir.dt.float32
F32R = mybir.dt.float32r
BF16 = mybir.dt.bfloat16
AF = mybir.ActivationFunctionType


def build_kernel():
    nc = bacc.Bacc(num_devices=N_CORES)

    xt_d = nc.dram_tensor("xt", [D, NTOK], F32, kind="ExternalInput")
    wqk_d = nc.dram_tensor("wqk", [D, 128], F32, kind="ExternalInput")
    bqk_d = nc.dram_tensor("bqk", [128, 1], F32, kind="ExternalInput")
    wv_d = nc.dram_tensor("wv", [D, DV], F32, kind="ExternalInput")
    bv_d = nc.dram_tensor("bv", [1, DV], F32, kind="ExternalInput")
    wo_d = nc.dram_tensor("wo", [D, D], F32, kind="ExternalInput")
    bo_d = nc.dram_tensor("bo", [1, D], F32, kind="ExternalInput")
    out_d = nc.dram_tensor("out", [TOKC, D], F32, kind="ExternalOutput")

    r32 = mybir.dt.float32r

    with tile.TileContext(nc) as tc, ExitStack() as ctx:
        pp = ctx.enter_context(tc.tile_pool(name="persist", bufs=1))
        dram = ctx.enter_context(tc.tile_pool(name="dram", bufs=1, space="DRAM"))

        # ---- persistent SBUF ----
        # Q^T/K^T in b-pairs: rows 0:64 = batch 2p, rows 64:128 = batch 2p+1
        qt = [pp.tile([128, S], BF16, tag=f"qt{p}", name=f"qt{p}") for p in range(2)]
        kt = [pp.tile([128, S], BF16, tag=f"kt{p}", name=f"kt{p}") for p in range(2)]
        # V-tilde: 64 token-chunks of [128 tok, 65] (col 64 = ones)
        vt = pp.tile([128, 64 * 65], BF16, tag="vt", name="vt")
        # Z^T (unnormalized) + denom row: [65, NTOK]
        zt = pp.tile([65, NTOK], BF16, tag="zt", name="zt")

        # weights
        wqk = [pp.tile([128, 128], r32, tag=f"wqk{i}", name=f"wqk{i}") for i in range(4)]
        wv = [pp.tile([128, DV], r32, tag=f"wv{i}", name=f"wv{i}") for i in range(4)]
        wo = [pp.tile([128, D], r32, tag=f"wo{i}", name=f"wo{i}") for i in range(4)]
        bqk = pp.tile([128, 1], F32, tag="bqk", name="bqk")
        bv = pp.tile([1, DV], r32, tag="bv", name="bv")
        bo = pp.tile([1, D], r32, tag="bo", name="bo")
        ones_row = pp.tile([1, 128], r32, tag="ones_row", name="ones_row")

        for i in range(4):
            nc.sync.dma_start(wqk[i][:], wqk_d[i * 128:(i + 1) * 128, :].bitcast(r32))
            nc.sync.dma_start(wv[i][:], wv_d[i * 128:(i + 1) * 128, :].bitcast(r32))
            nc.sync.dma_start(wo[i][:], wo_d[i * 128:(i + 1) * 128, :].bitcast(r32))
        nc.sync.dma_start(bqk[:], bqk_d[:])
        nc.sync.dma_start(bv[:], bv_d[:].bitcast(r32))
        nc.sync.dma_start(bo[:], bo_d[:].bitcast(r32))
        # memset is ISA-invalid for f32r tiles; stage ones in f32 and copy.
        onesf = pp.tile([128, 128], F32, tag="onesf", name="onesf")
        nc.vector.memset(onesf[:], 1.0)
        nc.vector.tensor_copy(ones_row[:], onesf[0:1, :])
        # ones column (col 64 of each 65-wide group) of V-tilde
        vt_ones = vt[:].rearrange("p (n c) -> p n c", c=65)[:, :, 64:65]
        nc.vector.tensor_copy(vt_ones, onesf[:, 0:64, None])

        # ================= Phase A: projections =================
        with (
            tc.tile_pool(name="xtp", bufs=2) as xp,
            tc.tile_pool(name="psA", bufs=2, space="PSUM") as psA,
        ):
            # b-inner order so the first 4 chunks cover (sc=0, t=0..3) of
            # every batch - lets attention start ~4x earlier
            for ck in [b * 4 + ssub for ssub in range(4) for b in range(4)]:
                b = ck // 4
                pair, row = b // 2, (b % 2) * 64
                xtile = [xp.tile([128, 512], r32, tag=f"xt{i}", name=f"xtile{i}") for i in range(4)]
                for i in range(4):
                    nc.sync.dma_start(
                        xtile[i][:],
                        xt_d[i * 128:(i + 1) * 128, ck * 512:(ck + 1) * 512].bitcast(r32),
                    )
                # Q^T | K^T (stacked 64+64) for this token chunk
                pqk = psA.tile([128, 512], F32, tag="pqk", name="pqk")
                for i in range(4):
                    nc.tensor.matmul(pqk[:], wqk[i][:], xtile[i][:],
                                     start=(i == 0), stop=(i == 3))
                scol = (ck % 4) * 512
                nc.scalar.activation(qt[pair][row:row + 64, scol:scol + 512],
                                     pqk[0:64, :], AF.Identity, bias=bqk[0:64, :])
                nc.scalar.activation(kt[pair][row:row + 64, scol:scol + 512],
                                     pqk[64:128, :], AF.Identity, bias=bqk[64:128, :])
                # V (natural layout) per 128-token subchunk, + ones-row bias fold
                for sub in range(4):
                    pv = psA.tile([128, DV], F32, tag="pv", name="pv")
                    for i in range(4):
                        nc.tensor.matmul(pv[:], xtile[i][:, sub * 128:(sub + 1) * 128],
                                         wv[i][:], start=(i == 0), stop=False)
                    nc.tensor.matmul(pv[:], ones_row[:], bv[:], start=False, stop=True)
                    tci = ck * 4 + sub  # global token-chunk index (b-major)
                    nc.vector.tensor_copy(vt[:, tci * 65:tci * 65 + 64], pv[:])

        # ================= Phase B: attention =================
        with (
            tc.tile_pool(name="wb", bufs=2) as wb,
            tc.tile_pool(name="psB", bufs=1, space="PSUM") as psB,
        ):
            # Software-pipelined over 64 global blocks g = sc*16 + t.
            # Per iteration: scores(g)+exp1(g) are emitted BEFORE the
            # DVE chain of g and exp2(g-1), so the ACT queue interleaves
            # exp1(g+1) ahead of exp2(g) and blocks overlap.
            NB = N_SC * N_TC
            pipe = {}  # g -> (e, p1) tiles

            def softmax_b(g):
                """scores(g) -> e(g) -> p1(g) tiles (no exp2 yet)."""
                sc, t = g // N_TC, g % N_TC
                scp = psB.tile([128, 4 * SC], F32, tag="scp", name="scp")
                for b in range(4):
                    pair, row = b // 2, (b % 2) * 64
                    nc.tensor.matmul(
                        scp[:, b * SC:(b + 1) * SC],
                        kt[pair][row:row + 64, t * TC:(t + 1) * TC],
                        qt[pair][row:row + 64, sc * SC:(sc + 1) * SC],
                        start=True, stop=True,
                    )
                # e = exp(scores/8) for all 4 b
                e = wb.tile([128, 4 * SC], BF16, tag="e", name="e")
                nc.scalar.activation(e[:], scp[:], AF.Exp, scale=0.125)
                # D = sum_b e ; r = 1/D (custom-DVE fast reciprocal keeps
                # ACT on the single exp table set - no table thrashing)
                t01 = wb.tile([128, 2 * SC], BF16, tag="t01", name="t01", bufs=1)
                nc.gpsimd.tensor_add(t01[:], e[:, 0:2 * SC], e[:, 2 * SC:4 * SC])
                dd = wb.tile([128, SC], F32, tag="dd", name="dd", bufs=1)
                nc.gpsimd.tensor_add(dd[:], t01[:, 0:SC], t01[:, SC:2 * SC])
                rf = wb.tile([128, SC], F32, tag="rf", name="rf", bufs=1)
                nc.vector.reciprocal_approx_fast(rf[:], dd[:])
                rr = wb.tile([128, SC], BF16, tag="rr", name="rr", bufs=1)
                nc.vector.tensor_copy(rr[:], rf[:])
                # p1 = e * r, one TT with r broadcast along the 4-b free dim
                p1 = wb.tile([128, 4 * SC], BF16, tag="p1", name="p1")
                nc.vector.tensor_mul(
                    p1[:].rearrange("p (b s) -> p b s", b=4),
                    e[:].rearrange("p (b s) -> p b s", b=4),
                    rr[:, None, :].broadcast_to([128, 4, SC]),
                )
                pipe[g] = p1

            def exp2_and_z(g, zacc):
                """exp2(g) + Z accumulation (ones-col -> sum_t in row 64)."""
                t = g % N_TC
                p1 = pipe.pop(g)
                q = wb.tile([128, 4 * SC], BF16, tag="q", name="q")
                nc.scalar.activation(q[:], p1[:], AF.Exp)
                for b in range(4):
                    tci = b * 16 + t
                    nc.tensor.matmul(
                        zacc[:, b * SC:(b + 1) * SC],
                        vt[:, tci * 65:(tci + 1) * 65],
                        q[:, b * SC:(b + 1) * SC],
                        start=(t == 0), stop=(t == N_TC - 1),
                    )

            zacc = None
            for g in range(NB + 1):
                if g < NB:
                    if g % N_TC == 0:
                        prev_zacc = zacc
                        zacc = psB.tile([65, 4 * SC], F32, tag="zacc", name="zacc")
                    softmax_b(g)
                if g >= 1:
                    gz = g - 1
                    za = prev_zacc if (g % N_TC == 0 and g < NB) else zacc
                    exp2_and_z(gz, za)
                    if gz % N_TC == N_TC - 1:
                        # evacuate Z^T (+denominator row) to bf16
                        sc_done = gz // N_TC
                        for b in range(4):
                            col = b * S + sc_done * SC
                            nc.vector.tensor_copy(zt[:, col:col + SC],
                                                  za[:, b * SC:(b + 1) * SC])

        # ================= AllToAll: re-shard by token =================
        a2a_in = dram.tile([N_CORES * 65, TOKC], BF16, tag="a2a_in", name="a2a_in")
        a2a_out = dram.tile([N_CORES * 65, TOKC], BF16, tag="a2a_out", name="a2a_out")
        for r in range(N_CORES):
            nc.sync.dma_start(a2a_in[r * 65:(r + 1) * 65, :],
                              zt[:, r * TOKC:(r + 1) * TOKC])
        nc.gpsimd.collective_compute(
            "AllToAll",
            mybir.AluOpType.bypass,
            replica_groups=[list(range(N_CORES))],
            ins=[a2a_in[:].opt()],
            outs=[a2a_out[:].opt()],
        )

        # ================= Phase C: output projection =================
        with (
            tc.tile_pool(name="wc", bufs=1) as wc,
            tc.tile_pool(name="oc", bufs=2) as oc,
            tc.tile_pool(name="psC", bufs=2, space="PSUM") as psC,
        ):
            zc = wc.tile([65, N_CORES * TOKC], BF16, tag="zc", name="zc")
            for j in range(N_CORES):
                nc.sync.dma_start(zc[:, j * TOKC:(j + 1) * TOKC],
                                  a2a_out[j * 65:(j + 1) * 65, :])
            # r2 = exp(-ln(denom)) per (head, token); broadcast to 64 partitions
            lden = wc.tile([1, N_CORES * TOKC], F32, tag="lden", name="lden")
            nc.scalar.activation(lden[:], zc[64:65, :], AF.Ln)
            rden = wc.tile([1, N_CORES * TOKC], BF16, tag="rden", name="rden")
            nc.scalar.activation(rden[:], lden[:], AF.Exp, scale=-1.0)
            rb = wc.tile([64, N_CORES * TOKC], BF16, tag="rb", name="rb")
            nc.gpsimd.partition_broadcast(rb[:], rden[:])
            # normalized Zc^T in hd-major pairs: tile i rows = heads 2i, 2i+1
            zcn = [wc.tile([128, TOKC], r32, tag=f"zcn{i}", name=f"zcn{i}") for i in range(4)]
            for j in range(N_CORES):
                nc.vector.tensor_mul(
                    zcn[j // 2][(j % 2) * 64:(j % 2) * 64 + 64, :],
                    zc[0:64, j * TOKC:(j + 1) * TOKC],
                    rb[:, j * TOKC:(j + 1) * TOKC],
                )
            for m in range(TOKC // 128):
                po = psC.tile([128, D], F32, tag="po", name="po")
                for i in range(4):
                    nc.tensor.matmul(po[:], zcn[i][:, m * 128:(m + 1) * 128],
                                     wo[i][:], start=(i == 0), stop=False)
                nc.tensor.matmul(po[:], ones_row[:], bo[:], start=False, stop=True)
                ot = oc.tile([128, D], F32, tag="ot", name="ot")
                nc.vector.tensor_copy(ot[:], po[:])
                nc.sync.dma_start(out_d[m * 128:(m + 1) * 128, :], ot[:])

    nc.compile()
    return nc


_NC_CACHE = None


def _get_nc():
    global _NC_CACHE
    if _NC_CACHE is None:
        _NC_CACHE = build_kernel()
    return _NC_CACHE


def kernel(X, WQ, bQ, WK, bK, WV, bV, WO, bO, _trace=False, _trace_kwargs=None):
    """Full inputs in, full output out. Shards internally across 8 cores."""
    X = np.asarray(X, dtype=np.float32)
    # [S,B,D] -> XT [D, NTOK] with b-major tokens (tok = b*S + s)
    xt = np.ascontiguousarray(X.transpose(2, 1, 0).reshape(D, NTOK))
    in_maps = []
    for h in range(N_CORES):
        wqk = np.ascontiguousarray(
            np.concatenate([WQ[h], WK[h]], axis=1), dtype=np.float32)
        bqk = np.ascontiguousarray(
            np.concatenate([bQ[h], bK[h]])[:, None], dtype=np.float32)
        in_maps.append({
            "xt": xt,
            "wqk": wqk,
            "bqk": bqk,
            "wv": np.ascontiguousarray(WV[h], dtype=np.float32),
            "bv": np.ascontiguousarray(bV[h][None, :], dtype=np.float32),
            "wo": np.ascontiguousarray(WO, dtype=np.float32),
            "bo": np.ascontiguousarray(bO[None, :], dtype=np.float32),
        })
    nc = _get_nc()
    res = run_bass_kernel_spmd(
        nc, in_maps, core_ids=list(range(N_CORES)),
        trace=_trace, **(_trace_kwargs or {}),
    )
    out = np.concatenate([res.results[c]["out"] for c in range(N_CORES)], axis=0)
    # [NTOK, D] b-major -> [S, B, D]
    full = out.reshape(B, S, D).transpose(1, 0, 2)
    if _trace:
        return np.ascontiguousarray(full), res
    return np.ascontiguousarray(full)


# revision 10
# speedup vs baseline: 1.2326x; 1.2326x over previous
"""Trainium2 Bass kernel for nn_Encoder_78795470012907.

Encoder layer: per-head Q/K/V projections, scores = QK^T/sqrt(dk),
double softmax (over batch axis, then over key axis), Z = pV, concat
heads, output projection. S=2048, B=4, D=512, H=8, dk=dv=64.

Sharding: head-parallel over 8 cores (core h owns head h) for the
attention; AllToAll re-shards by token for the output projection, so
each core emits a disjoint 1024-token slice of the output (host just
concatenates).

Layout notes (per core):
 - tokens are b-major: tok = b*2048 + s.
 - X is fed pre-transposed from host as XT [D, NTOK] (pure layout prep).
 - projections produce Q^T/K^T [dk, tok] stacked in b-pairs so the
   scores matmuls row-pack two batches into the 128x128 PE array.
 - scores are computed transposed ([t, s] tiles); the softmax over the
   key axis t rides the Z matmul via a ones-column appended to V
   (row 64 of the Z psum accumulates sum_t exp).
 - softmax over b: e=exp(s/8) -> D=sum_b e -> r=exp(-ln D) -> p1=e*r.
   ln+exp coexist in one ACT table set (no table thrashing);
   Reciprocal would force a table switch per tile.
"""

from contextlib import ExitStack

import numpy as np

import concourse.bass as bass
import concourse.tile as tile
from concourse import bacc, mybir
from concourse.bass_utils import run_bass_kernel_spmd

S, B, D = 2048, 4, 512
H, DK, DV = 8, 64, 64
N_CORES = 8
NTOK = S * B          # 8192 tokens, b-major
TOKC = NTOK // N_CORES  # 1024 tokens per core for the output slice
SC = 512              # s-chunk (columns of a scores^T tile)
TC = 128              # t-chunk (partitions of a scores^T tile)
N_SC = S // SC        # 4
N_TC = S // TC        # 16

F32 = mybir.dt.float32
F32R = mybir.dt.float32r
BF16 = mybir.dt.bfloat16
AF = mybir.ActivationFunctionType


def build_kernel():
    nc = bacc.Bacc(num_devices=N_CORES)

    xt_d = nc.dram_tensor("xt", [D, NTOK], F32, kind="ExternalInput")
    wqk_d = nc.dram_tensor("wqk", [D, 128], F32, kind="ExternalInput")
    bqk_d = nc.dram_tensor("bqk", [128, 1], F32, kind="ExternalInput")
    wv_d = nc.dram_tensor("wv", [D, DV], F32, kind="ExternalInput")
    bv_d = nc.dram_tensor("bv", [1, DV], F32, kind="ExternalInput")
    wo_d = nc.dram_tensor("wo", [D, D], F32, kind="ExternalInput")
    bo_d = nc.dram_tensor("bo", [1, D], F32, kind="ExternalInput")
    out_d = nc.dram_tensor("out", [TOKC, D], F32, kind="ExternalOutput")

    r32 = mybir.dt.float32r

    with tile.TileContext(nc) as tc, ExitStack() as ctx:
        pp = ctx.enter_context(tc.tile_pool(name="persist", bufs=1))
        dram = ctx.enter_context(tc.tile_pool(name="dram", bufs=1, space="DRAM"))

        # ---- persistent SBUF ----
        # Q^T/K^T in b-pairs: rows 0:64 = batch 2p, rows 64:128 = batch 2p+1
        qt = [pp.tile([128, S], BF16, tag=f"qt{p}", name=f"qt{p}") for p in range(2)]
        kt = [pp.tile([128, S], BF16, tag=f"kt{p}", name=f"kt{p}") for p in range(2)]
        # V-tilde: 64 token-chunks of [128 tok, 65] (col 64 = ones)
        vt = pp.tile([128, 64 * 65], BF16, tag="vt", name="vt")
        # Z^T (unnormalized) + denom row: [65, NTOK]
        zt = pp.tile([65, NTOK], BF16, tag="zt", name="zt")

        # weights
        wqk = [pp.tile([128, 128], r32, tag=f"wqk{i}", name=f"wqk{i}") for i in range(4)]
        wv = [pp.tile([128, DV], r32, tag=f"wv{i}", name=f"wv{i}") for i in range(4)]
        wo = [pp.tile([128, D], r32, tag=f"wo{i}", name=f"wo{i}") for i in range(4)]
        bqk = pp.tile([128, 1], F32, tag="bqk", name="bqk")
        bv = pp.tile([1, DV], r32, tag="bv", name="bv")
        bo = pp.tile([1, D], r32, tag="bo", name="bo")
        ones_row = pp.tile([1, 128], r32, tag="ones_row", name="ones_row")

        for i in range(4):
            nc.sync.dma_start(wqk[i][:], wqk_d[i * 128:(i + 1) * 128, :].bitcast(r32))
            nc.sync.dma_start(wv[i][:], wv_d[i * 128:(i + 1) * 128, :].bitcast(r32))
            nc.sync.dma_start(wo[i][:], wo_d[i * 128:(i + 1) * 128, :].bitcast(r32))
        nc.sync.dma_start(bqk[:], bqk_d[:])
        nc.sync.dma_start(bv[:], bv_d[:].bitcast(r32))
        nc.sync.dma_start(bo[:], bo_d[:].bitcast(r32))
        # memset is ISA-invalid for f32r tiles; stage ones in f32 and copy.
        onesf = pp.tile([128, 128], F32, tag="onesf", name="onesf")
        nc.vector.memset(onesf[:], 1.0)
        nc.vector.tensor_copy(ones_row[:], onesf[0:1, :])
        # ones column (col 64 of each 65-wide group) of V-tilde
        vt_ones = vt[:].rearrange("p (n c) -> p n c", c=65)[:, :, 64:65]
        nc.vector.tensor_copy(vt_ones, onesf[:, 0:64, None])

        # ================= Phase A: projections =================
        with (
            tc.tile_pool(name="xtp", bufs=2) as xp,
            tc.tile_pool(name="psA", bufs=2, space="PSUM") as psA,
        ):
            # b-inner order so the first 4 chunks cover (sc=0, t=0..3) of
            # every batch - lets attention start ~4x earlier
            for ck in [b * 4 + ssub for ssub in range(4) for b in range(4)]:
                b = ck // 4
                pair, row = b // 2, (b % 2) * 64
                xtile = [xp.tile([128, 512], r32, tag=f"xt{i}", name=f"xtile{i}") for i in range(4)]
                for i in range(4):
                    nc.sync.dma_start(
                        xtile[i][:],
                        xt_d[i * 128:(i + 1) * 128, ck * 512:(ck + 1) * 512].bitcast(r32),
                    )
                # Q^T | K^T (stacked 64+64) for this token chunk
                pqk = psA.tile([128, 512], F32, tag="pqk", name="pqk")
                for i in range(4):
                    nc.tensor.matmul(pqk[:], wqk[i][:], xtile[i][:],
                                     start=(i == 0), stop=(i == 3))
                scol = (ck % 4) * 512
                nc.scalar.activation(qt[pair][row:row + 64, scol:scol + 512],
                                     pqk[0:64, :], AF.Identity, bias=bqk[0:64, :])
                nc.scalar.activation(kt[pair][row:row + 64, scol:scol + 512],
                                     pqk[64:128, :], AF.Identity, bias=bqk[64:128, :])
                # V (natural layout) per 128-token subchunk, + ones-row bias fold
                for sub in range(4):
                    pv = psA.tile([128, DV], F32, tag="pv", name="pv")
                    for i in range(4):
                        nc.tensor.matmul(pv[:], xtile[i][:, sub * 128:(sub + 1) * 128],
                                         wv[i][:], start=(i == 0), stop=False)
                    nc.tensor.matmul(pv[:], ones_row[:], bv[:], start=False, stop=True)
                    tci = ck * 4 + sub  # global token-chunk index (b-major)
                    nc.vector.tensor_copy(vt[:, tci * 65:tci * 65 + 64], pv[:])

        # ================= Phase B: attention =================
        with (
            tc.tile_pool(name="wb", bufs=2) as wb,
            tc.tile_pool(name="psB", bufs=1, space="PSUM") as psB,
        ):
            # Software-pipelined over 64 global blocks g = sc*16 + t.
            # Per iteration: scores(g)+exp1(g) are emitted BEFORE the
            # DVE chain of g and exp2(g-1), so the ACT queue interleaves
            # exp1(g+1) ahead of exp2(g) and blocks overlap.
            NB = N_SC * N_TC
            pipe = {}  # g -> (e, p1) tiles

            def softmax_b(g):
                """scores(g) -> e(g) -> p1(g) tiles (no exp2 yet)."""
                sc, t = g // N_TC, g % N_TC
                scp = psB.tile([128, 4 * SC], F32, tag="scp", name="scp")
                for b in range(4):
                    pair, row = b // 2, (b % 2) * 64
                    nc.tensor.matmul(
                        scp[:, b * SC:(b + 1) * SC],
                        kt[pair][row:row + 64, t * TC:(t + 1) * TC],
                        qt[pair][row:row + 64, sc * SC:(sc + 1) * SC],
                        start=True, stop=True,
                    )
                # e = exp(scores/8) for all 4 b
                e = wb.tile([128, 4 * SC], BF16, tag="e", name="e")
                nc.scalar.activation(e[:], scp[:], AF.Exp, scale=0.125)
                # D = sum_b e ; r = 1/D (custom-DVE fast reciprocal keeps
                # ACT on the single exp table set - no table thrashing)
                t01 = wb.tile([128, 2 * SC], BF16, tag="t01", name="t01", bufs=1)
                nc.gpsimd.tensor_add(t01[:], e[:, 0:2 * SC], e[:, 2 * SC:4 * SC])
                dd = wb.tile([128, SC], F32, tag="dd", name="dd", bufs=1)
                nc.vector.tensor_add(dd[:], t01[:, 0:SC], t01[:, SC:2 * SC])
                rf = wb.tile([128, SC], F32, tag="rf", name="rf", bufs=1)
                nc.vector.reciprocal_approx_fast(rf[:], dd[:])
                rr = wb.tile([128, SC], BF16, tag="rr", name="rr", bufs=1)
                nc.vector.tensor_copy(rr[:], rf[:])
                # p1 = e * r, one TT with r broadcast along the 4-b free dim
                p1 = wb.tile([128, 4 * SC], BF16, tag="p1", name="p1")
                nc.vector.tensor_mul(
                    p1[:].rearrange("p (b s) -> p b s", b=4),
                    e[:].rearrange("p (b s) -> p b s", b=4),
                    rr[:, None, :].broadcast_to([128, 4, SC]),
                )
                pipe[g] = p1

            def exp2_and_z(g, zacc):
                """exp2(g) + Z accumulation (ones-col -> sum_t in row 64)."""
                t = g % N_TC
                p1 = pipe.pop(g)
                q = wb.tile([128, 4 * SC], BF16, tag="q", name="q")
                nc.scalar.activation(q[:], p1[:], AF.Exp)
                for b in range(4):
                    tci = b * 16 + t
                    nc.tensor.matmul(
                        zacc[:, b * SC:(b + 1) * SC],
                        vt[:, tci * 65:(tci + 1) * 65],
                        q[:, b * SC:(b + 1) * SC],
                        start=(t == 0), stop=(t == N_TC - 1),
                    )

            zacc = None
            for g in range(NB + 1):
                if g < NB:
                    if g % N_TC == 0:
                        prev_zacc = zacc
                        zacc = psB.tile([65, 4 * SC], F32, tag="zacc", name="zacc")
                    softmax_b(g)
                if g >= 1:
                    gz = g - 1
                    za = prev_zacc if (g % N_TC == 0 and g < NB) else zacc
                    exp2_and_z(gz, za)
                    if gz % N_TC == N_TC - 1:
                        # evacuate Z^T (+denominator row) to bf16
                        sc_done = gz // N_TC
                        for b in range(4):
                            col = b * S + sc_done * SC
                            nc.vector.tensor_copy(zt[:, col:col + SC],
                                                  za[:, b * SC:(b + 1) * SC])

        # ================= AllToAll: re-shard by token =================
        a2a_in = dram.tile([N_CORES * 65, TOKC], BF16, tag="a2a_in", name="a2a_in")
        a2a_out = dram.tile([N_CORES * 65, TOKC], BF16, tag="a2a_out", name="a2a_out")
        for r in range(N_CORES):
            nc.sync.dma_start(a2a_in[r * 65:(r + 1) * 65, :],
                              zt[:, r * TOKC:(r + 1) * TOKC])
        nc.gpsimd.collective_compute(
            "AllToAll",
            mybir.AluOpType.bypass,
            replica_groups=[list(range(N_CORES))],
            ins=[a2a_in[:].opt()],
            outs=[a2a_out[:].opt()],
        )

        # ================= Phase C: output projection =================
        with (
            tc.tile_pool(name="wc", bufs=1) as wc,
            tc.tile_pool(name="oc", bufs=2) as oc,
            tc.tile_pool(name="psC", bufs=2, space="PSUM") as psC,
        ):
            zc = wc.tile([65, N_CORES * TOKC], BF16, tag="zc", name="zc")
            for j in range(N_CORES):
                nc.sync.dma_start(zc[:, j * TOKC:(j + 1) * TOKC],
                                  a2a_out[j * 65:(j + 1) * 65, :])
            # r2 = exp(-ln(denom)) per (head, token); broadcast to 64 partitions
            lden = wc.tile([1, N_CORES * TOKC], F32, tag="lden", name="lden")
            nc.scalar.activation(lden[:], zc[64:65, :], AF.Ln)
            rden = wc.tile([1, N_CORES * TOKC], BF16, tag="rden", name="rden")
            nc.scalar.activation(rden[:], lden[:], AF.Exp, scale=-1.0)
            rb = wc.tile([64, N_CORES * TOKC], BF16, tag="rb", name="rb")
            nc.gpsimd.partition_broadcast(rb[:], rden[:])
            # normalized Zc^T in hd-major pairs: tile i rows = heads 2i, 2i+1
            zcn = [wc.tile([128, TOKC], r32, tag=f"zcn{i}", name=f"zcn{i}") for i in range(4)]
            for j in range(N_CORES):
                nc.vector.tensor_mul(
                    zcn[j // 2][(j % 2) * 64:(j % 2) * 64 + 64, :],
                    zc[0:64, j * TOKC:(j + 1) * TOKC],
                    rb[:, j * TOKC:(j + 1) * TOKC],
                )
            for m in range(TOKC // 128):
                po = psC.tile([128, D], F32, tag="po", name="po")
                for i in range(4):
                    nc.tensor.matmul(po[:], zcn[i][:, m * 128:(m + 1) * 128],
                                     wo[i][:], start=(i == 0), stop=False)
                nc.tensor.matmul(po[:], ones_row[:], bo[:], start=False, stop=True)
                ot = oc.tile([128, D], F32, tag="ot", name="ot")
                nc.vector.tensor_copy(ot[:], po[:])
                nc.sync.dma_start(out_d[m * 128:(m + 1) * 128, :], ot[:])

    nc.compile()
    return nc


_NC_CACHE = None


def _get_nc():
    global _NC_CACHE
    if _NC_CACHE is None:
        _NC_CACHE = build_kernel()
    return _NC_CACHE


def kernel(X, WQ, bQ, WK, bK, WV, bV, WO, bO, _trace=False, _trace_kwargs=None):
    """Full inputs in, full output out. Shards internally across 8 cores."""
    X = np.asarray(X, dtype=np.float32)
    # [S,B,D] -> XT [D, NTOK] with b-major tokens (tok = b*S + s)
    xt = np.ascontiguousarray(X.transpose(2, 1, 0).reshape(D, NTOK))
    in_maps = []
    for h in range(N_CORES):
        wqk = np.ascontiguousarray(
            np.concatenate([WQ[h], WK[h]], axis=1), dtype=np.float32)
        bqk = np.ascontiguousarray(
            np.concatenate([bQ[h], bK[h]])[:, None], dtype=np.float32)
        in_maps.append({
            "xt": xt,
            "wqk": wqk,
            "bqk": bqk,
            "wv": np.ascontiguousarray(WV[h], dtype=np.float32),
            "bv": np.ascontiguousarray(bV[h][None, :], dtype=np.float32),
            "wo": np.ascontiguousarray(WO, dtype=np.float32),
            "bo": np.ascontiguousarray(bO[None, :], dtype=np.float32),
        })
    nc = _get_nc()
    res = run_bass_kernel_spmd(
        nc, in_maps, core_ids=list(range(N_CORES)),
        trace=_trace, **(_trace_kwargs or {}),
    )
    out = np.concatenate([res.results[c]["out"] for c in range(N_CORES)], axis=0)
    # [NTOK, D] b-major -> [S, B, D]
    full = out.reshape(B, S, D).transpose(1, 0, 2)
    if _trace:
        return np.ascontiguousarray(full), res
    return np.ascontiguousarray(full)


# revision 11
# speedup vs baseline: 1.2786x; 1.0373x over previous
"""Trainium2 Bass kernel for nn_Encoder_78795470012907.

Encoder layer: per-head Q/K/V projections, scores = QK^T/sqrt(dk),
double softmax (over batch axis, then over key axis), Z = pV, concat
heads, output projection. S=2048, B=4, D=512, H=8, dk=dv=64.

Sharding: head-parallel over 8 cores (core h owns head h) for the
attention; AllToAll re-shards by token for the output projection, so
each core emits a disjoint 1024-token slice of the output (host just
concatenates).

Layout notes (per core):
 - tokens are b-major: tok = b*2048 + s.
 - X is fed pre-transposed from host as XT [D, NTOK] (pure layout prep).
 - projections produce Q^T/K^T [dk, tok] stacked in b-pairs so the
   scores matmuls row-pack two batches into the 128x128 PE array.
 - scores are computed transposed ([t, s] tiles); the softmax over the
   key axis t rides the Z matmul via a ones-column appended to V
   (row 64 of the Z psum accumulates sum_t exp).
 - softmax over b: e=exp(s/8) -> D=sum_b e -> r=exp(-ln D) -> p1=e*r.
   ln+exp coexist in one ACT table set (no table thrashing);
   Reciprocal would force a table switch per tile.
"""

from contextlib import ExitStack

import numpy as np

import concourse.bass as bass
import concourse.tile as tile
from concourse import bacc, mybir
from concourse.bass_utils import run_bass_kernel_spmd

S, B, D = 2048, 4, 512
H, DK, DV = 8, 64, 64
N_CORES = 8
NTOK = S * B          # 8192 tokens, b-major
TOKC = NTOK // N_CORES  # 1024 tokens per core for the output slice
SC = 512              # s-chunk (columns of a scores^T tile)
TC = 128              # t-chunk (partitions of a scores^T tile)
N_SC = S // SC        # 4
N_TC = S // TC        # 16

F32 = mybir.dt.float32
F32R = mybir.dt.float32r
BF16 = mybir.dt.bfloat16
AF = mybir.ActivationFunctionType


def build_kernel():
    nc = bacc.Bacc(num_devices=N_CORES)

    xt_d = nc.dram_tensor("xt", [D, NTOK], F32, kind="ExternalInput")
    wqk_d = nc.dram_tensor("wqk", [D, 128], F32, kind="ExternalInput")
    bqk_d = nc.dram_tensor("bqk", [128, 1], F32, kind="ExternalInput")
    wv_d = nc.dram_tensor("wv", [D, DV], F32, kind="ExternalInput")
    bv_d = nc.dram_tensor("bv", [1, DV], F32, kind="ExternalInput")
    wo_d = nc.dram_tensor("wo", [D, D], F32, kind="ExternalInput")
    bo_d = nc.dram_tensor("bo", [1, D], F32, kind="ExternalInput")
    out_d = nc.dram_tensor("out", [TOKC, D], F32, kind="ExternalOutput")

    r32 = mybir.dt.float32r

    with tile.TileContext(nc) as tc, ExitStack() as ctx:
        pp = ctx.enter_context(tc.tile_pool(name="persist", bufs=1))
        dram = ctx.enter_context(tc.tile_pool(name="dram", bufs=1, space="DRAM"))

        # ---- persistent SBUF ----
        # Q^T/K^T in b-pairs: rows 0:64 = batch 2p, rows 64:128 = batch 2p+1
        qt = [pp.tile([128, S], BF16, tag=f"qt{p}", name=f"qt{p}") for p in range(2)]
        kt = [pp.tile([128, S], BF16, tag=f"kt{p}", name=f"kt{p}") for p in range(2)]
        # V-tilde: 64 token-chunks of [128 tok, 65] (col 64 = ones)
        vt = pp.tile([128, 64 * 65], BF16, tag="vt", name="vt")
        # Z^T (unnormalized) + denom row: [65, NTOK]
        zt = pp.tile([65, NTOK], BF16, tag="zt", name="zt")

        # weights
        wqk = [pp.tile([128, 128], r32, tag=f"wqk{i}", name=f"wqk{i}") for i in range(4)]
        wv = [pp.tile([128, DV], r32, tag=f"wv{i}", name=f"wv{i}") for i in range(4)]
        wo = [pp.tile([128, D], r32, tag=f"wo{i}", name=f"wo{i}") for i in range(4)]
        bqk = pp.tile([128, 1], F32, tag="bqk", name="bqk")
        bv = pp.tile([1, DV], r32, tag="bv", name="bv")
        bo = pp.tile([1, D], r32, tag="bo", name="bo")
        ones_row = pp.tile([1, 128], r32, tag="ones_row", name="ones_row")

        for i in range(4):
            nc.sync.dma_start(wqk[i][:], wqk_d[i * 128:(i + 1) * 128, :].bitcast(r32))
            nc.sync.dma_start(wv[i][:], wv_d[i * 128:(i + 1) * 128, :].bitcast(r32))
            nc.sync.dma_start(wo[i][:], wo_d[i * 128:(i + 1) * 128, :].bitcast(r32))
        nc.sync.dma_start(bqk[:], bqk_d[:])
        nc.sync.dma_start(bv[:], bv_d[:].bitcast(r32))
        nc.sync.dma_start(bo[:], bo_d[:].bitcast(r32))
        # memset is ISA-invalid for f32r tiles; stage ones in f32 and copy.
        onesf = pp.tile([128, 128], F32, tag="onesf", name="onesf")
        nc.vector.memset(onesf[:], 1.0)
        nc.vector.tensor_copy(ones_row[:], onesf[0:1, :])
        # ones column (col 64 of each 65-wide group) of V-tilde
        vt_ones = vt[:].rearrange("p (n c) -> p n c", c=65)[:, :, 64:65]
        nc.vector.tensor_copy(vt_ones, onesf[:, 0:64, None])

        # ================= Phase A: projections =================
        with (
            tc.tile_pool(name="xtp", bufs=2) as xp,
            tc.tile_pool(name="psA", bufs=2, space="PSUM") as psA,
        ):
            # b-inner order so the first 4 chunks cover (sc=0, t=0..3) of
            # every batch - lets attention start ~4x earlier
            for ck in [b * 4 + ssub for ssub in range(4) for b in range(4)]:
                b = ck // 4
                pair, row = b // 2, (b % 2) * 64
                xtile = [xp.tile([128, 512], r32, tag=f"xt{i}", name=f"xtile{i}") for i in range(4)]
                for i in range(4):
                    nc.sync.dma_start(
                        xtile[i][:],
                        xt_d[i * 128:(i + 1) * 128, ck * 512:(ck + 1) * 512].bitcast(r32),
                    )
                # Q^T | K^T (stacked 64+64) for this token chunk
                pqk = psA.tile([128, 512], F32, tag="pqk", name="pqk")
                for i in range(4):
                    nc.tensor.matmul(pqk[:], wqk[i][:], xtile[i][:],
                                     start=(i == 0), stop=(i == 3))
                scol = (ck % 4) * 512
                nc.scalar.activation(qt[pair][row:row + 64, scol:scol + 512],
                                     pqk[0:64, :], AF.Identity, bias=bqk[0:64, :])
                nc.scalar.activation(kt[pair][row:row + 64, scol:scol + 512],
                                     pqk[64:128, :], AF.Identity, bias=bqk[64:128, :])
                # V (natural layout) per 128-token subchunk, + ones-row bias fold
                for sub in range(4):
                    pv = psA.tile([128, DV], F32, tag="pv", name="pv")
                    for i in range(4):
                        nc.tensor.matmul(pv[:], xtile[i][:, sub * 128:(sub + 1) * 128],
                                         wv[i][:], start=(i == 0), stop=False)
                    nc.tensor.matmul(pv[:], ones_row[:], bv[:], start=False, stop=True)
                    tci = ck * 4 + sub  # global token-chunk index (b-major)
                    nc.vector.tensor_copy(vt[:, tci * 65:tci * 65 + 64], pv[:])

        # ================= Phase B: attention =================
        with (
            tc.tile_pool(name="wb", bufs=2) as wb,
            tc.tile_pool(name="psB", bufs=1, space="PSUM") as psB,
        ):
            # Software-pipelined over 64 global blocks g = sc*16 + t.
            # Per iteration: scores(g)+exp1(g) are emitted BEFORE the
            # DVE chain of g and exp2(g-1), so the ACT queue interleaves
            # exp1(g+1) ahead of exp2(g) and blocks overlap.
            NB = N_SC * N_TC
            pipe = {}  # g -> (e, p1) tiles

            def softmax_b(g):
                """scores(g) -> e(g) -> p1(g) tiles (no exp2 yet)."""
                sc, t = g // N_TC, g % N_TC
                scp = psB.tile([128, 4 * SC], F32, tag="scp", name="scp")
                for b in range(4):
                    pair, row = b // 2, (b % 2) * 64
                    nc.tensor.matmul(
                        scp[:, b * SC:(b + 1) * SC],
                        kt[pair][row:row + 64, t * TC:(t + 1) * TC],
                        qt[pair][row:row + 64, sc * SC:(sc + 1) * SC],
                        start=True, stop=True,
                    )
                # e = exp(scores/8) for all 4 b
                e = wb.tile([128, 4 * SC], BF16, tag="e", name="e")
                nc.scalar.activation(e[:], scp[:], AF.Exp, scale=0.125)
                # D = sum_b e ; r = 1/D (custom-DVE fast reciprocal keeps
                # ACT on the single exp table set - no table thrashing)
                t01 = wb.tile([128, 2 * SC], BF16, tag="t01", name="t01", bufs=1)
                nc.gpsimd.tensor_add(t01[:], e[:, 0:2 * SC], e[:, 2 * SC:4 * SC])
                dd = wb.tile([128, SC], F32, tag="dd", name="dd", bufs=1)
                nc.vector.tensor_add(dd[:], t01[:, 0:SC], t01[:, SC:2 * SC])
                rf = wb.tile([128, SC], F32, tag="rf", name="rf", bufs=1)
                nc.vector.reciprocal_approx_fast(rf[:], dd[:])
                # p1 = e * r, one TT with r broadcast along the 4-b free dim
                p1 = wb.tile([128, 4 * SC], BF16, tag="p1", name="p1")
                nc.vector.tensor_mul(
                    p1[:].rearrange("p (b s) -> p b s", b=4),
                    e[:].rearrange("p (b s) -> p b s", b=4),
                    rf[:, None, :].broadcast_to([128, 4, SC]),
                )
                pipe[g] = p1

            def exp2_and_z(g, zacc):
                """exp2(g) + Z accumulation (ones-col -> sum_t in row 64)."""
                t = g % N_TC
                p1 = pipe.pop(g)
                q = wb.tile([128, 4 * SC], BF16, tag="q", name="q")
                nc.scalar.activation(q[:], p1[:], AF.Exp)
                for b in range(4):
                    tci = b * 16 + t
                    nc.tensor.matmul(
                        zacc[:, b * SC:(b + 1) * SC],
                        vt[:, tci * 65:(tci + 1) * 65],
                        q[:, b * SC:(b + 1) * SC],
                        start=(t == 0), stop=(t == N_TC - 1),
                    )

            a2a_in_h = [dram.tile([N_CORES * 65, 512], BF16, tag=f"a2a_in{q}",
                                  name=f"a2a_in{q}") for q in range(2)]
            a2a_out_h = [dram.tile([N_CORES * 65, 512], BF16, tag=f"a2a_out{q}",
                                   name=f"a2a_out{q}") for q in range(2)]

            def emit_a2a(q):
                # chunk r = my head's Z^T cols for core r's half-q tokens:
                # tok = (r//2)*S + q*1024 + (r%2)*512 ... +512
                for r in range(N_CORES):
                    col = (r // 2) * S + q * 1024 + (r % 2) * 512
                    nc.sync.dma_start(a2a_in_h[q][r * 65:(r + 1) * 65, :],
                                      zt[:, col:col + 512])
                nc.gpsimd.collective_compute(
                    "AllToAll",
                    mybir.AluOpType.bypass,
                    replica_groups=[list(range(N_CORES))],
                    ins=[a2a_in_h[q][:].opt()],
                    outs=[a2a_out_h[q][:].opt()],
                )

            zacc = None
            for g in range(NB + 1):
                if g < NB:
                    if g % N_TC == 0:
                        prev_zacc = zacc
                        zacc = psB.tile([65, 4 * SC], F32, tag="zacc", name="zacc")
                    softmax_b(g)
                if g >= 1:
                    gz = g - 1
                    za = prev_zacc if (g % N_TC == 0 and g < NB) else zacc
                    exp2_and_z(gz, za)
                    if gz % N_TC == N_TC - 1:
                        # evacuate Z^T (+denominator row) to bf16
                        sc_done = gz // N_TC
                        for b in range(4):
                            col = b * S + sc_done * SC
                            nc.vector.tensor_copy(zt[:, col:col + SC],
                                                  za[:, b * SC:(b + 1) * SC])
                        if sc_done == 1:
                            emit_a2a(0)  # overlaps remaining attention
                        elif sc_done == 3:
                            emit_a2a(1)

        # ================= Phase C: output projection =================
        with (
            tc.tile_pool(name="wc", bufs=1) as wc,
            tc.tile_pool(name="oc", bufs=2) as oc,
            tc.tile_pool(name="psC", bufs=2, space="PSUM") as psC,
        ):
            HT = 512  # tokens per half
            for q in range(2):
                zc = wc.tile([65, N_CORES * HT], BF16, tag="zc", name="zc", bufs=2)
                for j in range(N_CORES):
                    nc.sync.dma_start(zc[:, j * HT:(j + 1) * HT],
                                      a2a_out_h[q][j * 65:(j + 1) * 65, :])
                # r2 = exp(-ln(denom)) per (head, token); bcast to 64 rows
                lden = wc.tile([1, N_CORES * HT], F32, tag="lden", name="lden", bufs=2)
                nc.scalar.activation(lden[:], zc[64:65, :], AF.Ln)
                rden = wc.tile([1, N_CORES * HT], BF16, tag="rden", name="rden", bufs=2)
                nc.scalar.activation(rden[:], lden[:], AF.Exp, scale=-1.0)
                rb = wc.tile([64, N_CORES * HT], BF16, tag="rb", name="rb", bufs=2)
                nc.gpsimd.partition_broadcast(rb[:], rden[:])
                # normalized Zc^T in hd-major pairs: tile i = heads 2i, 2i+1
                zcn = [wc.tile([128, HT], r32, tag=f"zcn{i}", name=f"zcn{i}", bufs=2)
                       for i in range(4)]
                for j in range(N_CORES):
                    nc.vector.tensor_mul(
                        zcn[j // 2][(j % 2) * 64:(j % 2) * 64 + 64, :],
                        zc[0:64, j * HT:(j + 1) * HT],
                        rb[:, j * HT:(j + 1) * HT],
                    )
                for m in range(HT // 128):
                    po = psC.tile([128, D], F32, tag="po", name="po")
                    for i in range(4):
                        nc.tensor.matmul(po[:], zcn[i][:, m * 128:(m + 1) * 128],
                                         wo[i][:], start=(i == 0), stop=False)
                    nc.tensor.matmul(po[:], ones_row[:], bo[:], start=False,
                                     stop=True)
                    ot = oc.tile([128, D], F32, tag="ot", name="ot")
                    nc.vector.tensor_copy(ot[:], po[:])
                    row = q * HT + m * 128
                    nc.sync.dma_start(out_d[row:row + 128, :], ot[:])

    nc.compile()
    return nc


_NC_CACHE = None


def _get_nc():
    global _NC_CACHE
    if _NC_CACHE is None:
        _NC_CACHE = build_kernel()
    return _NC_CACHE


def kernel(X, WQ, bQ, WK, bK, WV, bV, WO, bO, _trace=False, _trace_kwargs=None):
    """Full inputs in, full output out. Shards internally across 8 cores."""
    X = np.asarray(X, dtype=np.float32)
    # [S,B,D] -> XT [D, NTOK] with b-major tokens (tok = b*S + s)
    xt = np.ascontiguousarray(X.transpose(2, 1, 0).reshape(D, NTOK))
    in_maps = []
    for h in range(N_CORES):
        wqk = np.ascontiguousarray(
            np.concatenate([WQ[h], WK[h]], axis=1), dtype=np.float32)
        bqk = np.ascontiguousarray(
            np.concatenate([bQ[h], bK[h]])[:, None], dtype=np.float32)
        in_maps.append({
            "xt": xt,
            "wqk": wqk,
            "bqk": bqk,
            "wv": np.ascontiguousarray(WV[h], dtype=np.float32),
            "bv": np.ascontiguousarray(bV[h][None, :], dtype=np.float32),
            "wo": np.ascontiguousarray(WO, dtype=np.float32),
            "bo": np.ascontiguousarray(bO[None, :], dtype=np.float32),
        })
    nc = _get_nc()
    res = run_bass_kernel_spmd(
        nc, in_maps, core_ids=list(range(N_CORES)),
        trace=_trace, **(_trace_kwargs or {}),
    )
    # core c rows: [0:512] = tokens (c//2)*S + (c%2)*512 .. ; [512:1024] same + 1024
    fullb = np.empty((B, S, D), dtype=np.float32)
    for c in range(N_CORES):
        oc = res.results[c]["out"]
        b, off = c // 2, (c % 2) * 512
        fullb[b, off:off + 512] = oc[0:512]
        fullb[b, 1024 + off:1024 + off + 512] = oc[512:1024]
    full = fullb.transpose(1, 0, 2)
    if _trace:
        return np.ascontiguousarray(full), res
    return np.ascontiguousarray(full)


# revision 14
# speedup vs baseline: 1.3088x; 1.0237x over previous
"""Trainium2 Bass kernel for nn_Encoder_78795470012907.

Encoder layer: per-head Q/K/V projections, scores = QK^T/sqrt(dk),
double softmax (over batch axis, then over key axis), Z = pV, concat
heads, output projection. S=2048, B=4, D=512, H=8, dk=dv=64.

Sharding: head-parallel over 8 cores (core h owns head h) for the
attention; AllToAll re-shards by token for the output projection, so
each core emits a disjoint 1024-token slice of the output (host just
concatenates).

Layout notes (per core):
 - tokens are b-major: tok = b*2048 + s.
 - X is fed pre-transposed from host as XT [D, NTOK] (pure layout prep).
 - projections produce Q^T/K^T [dk, tok] stacked in b-pairs so the
   scores matmuls row-pack two batches into the 128x128 PE array.
 - scores are computed transposed ([t, s] tiles); the softmax over the
   key axis t rides the Z matmul via a ones-column appended to V
   (row 64 of the Z psum accumulates sum_t exp).
 - softmax over b: e=exp(s/8) -> D=sum_b e -> r=exp(-ln D) -> p1=e*r.
   ln+exp coexist in one ACT table set (no table thrashing);
   Reciprocal would force a table switch per tile.
"""

from contextlib import ExitStack

import numpy as np

import concourse.bass as bass
import concourse.tile as tile
from concourse import bacc, mybir
from concourse.bass_utils import run_bass_kernel_spmd

S, B, D = 2048, 4, 512
H, DK, DV = 8, 64, 64
N_CORES = 8
NTOK = S * B          # 8192 tokens, b-major
TOKC = NTOK // N_CORES  # 1024 tokens per core for the output slice
SC = 512              # s-chunk (columns of a scores^T tile)
TC = 128              # t-chunk (partitions of a scores^T tile)
N_SC = S // SC        # 4
N_TC = S // TC        # 16

F32 = mybir.dt.float32
F32R = mybir.dt.float32r
BF16 = mybir.dt.bfloat16
AF = mybir.ActivationFunctionType


def build_kernel():
    nc = bacc.Bacc(num_devices=N_CORES)

    xt_d = nc.dram_tensor("xt", [D, NTOK], F32, kind="ExternalInput")
    wqk_d = nc.dram_tensor("wqk", [D, 128], F32, kind="ExternalInput")
    bqk_d = nc.dram_tensor("bqk", [128, 1], F32, kind="ExternalInput")
    wv_d = nc.dram_tensor("wv", [D, DV], F32, kind="ExternalInput")
    bv_d = nc.dram_tensor("bv", [1, DV], F32, kind="ExternalInput")
    wo_d = nc.dram_tensor("wo", [D, D], F32, kind="ExternalInput")
    bo_d = nc.dram_tensor("bo", [1, D], F32, kind="ExternalInput")
    out_d = nc.dram_tensor("out", [TOKC, D], F32, kind="ExternalOutput")

    r32 = mybir.dt.float32r

    with tile.TileContext(nc) as tc, ExitStack() as ctx:
        pp = ctx.enter_context(tc.tile_pool(name="persist", bufs=1))
        dram = ctx.enter_context(tc.tile_pool(name="dram", bufs=1, space="DRAM"))

        # ---- persistent SBUF ----
        # Q^T/K^T in b-pairs: rows 0:64 = batch 2p, rows 64:128 = batch 2p+1
        qt = [pp.tile([128, S], BF16, tag=f"qt{p}", name=f"qt{p}") for p in range(2)]
        kt = [pp.tile([128, S], BF16, tag=f"kt{p}", name=f"kt{p}") for p in range(2)]
        # V-tilde: 64 token-chunks of [128 tok, 65] (col 64 = ones)
        vt = pp.tile([128, 64 * 65], BF16, tag="vt", name="vt")
        # Z^T (unnormalized) + denom row: [65, NTOK]
        zt = pp.tile([65, NTOK], BF16, tag="zt", name="zt")

        # weights
        wqk = [pp.tile([128, 128], r32, tag=f"wqk{i}", name=f"wqk{i}") for i in range(4)]
        wv = [pp.tile([128, DV], r32, tag=f"wv{i}", name=f"wv{i}") for i in range(4)]
        wo = [pp.tile([128, D], r32, tag=f"wo{i}", name=f"wo{i}") for i in range(4)]
        bqk = pp.tile([128, 1], F32, tag="bqk", name="bqk")
        bv = pp.tile([1, DV], r32, tag="bv", name="bv")
        bo = pp.tile([1, D], r32, tag="bo", name="bo")
        ones_row = pp.tile([1, 128], r32, tag="ones_row", name="ones_row")

        for i in range(4):
            nc.sync.dma_start(wqk[i][:], wqk_d[i * 128:(i + 1) * 128, :].bitcast(r32))
            nc.sync.dma_start(wv[i][:], wv_d[i * 128:(i + 1) * 128, :].bitcast(r32))
            nc.sync.dma_start(wo[i][:], wo_d[i * 128:(i + 1) * 128, :].bitcast(r32))
        nc.sync.dma_start(bqk[:], bqk_d[:])
        nc.sync.dma_start(bv[:], bv_d[:].bitcast(r32))
        nc.sync.dma_start(bo[:], bo_d[:].bitcast(r32))
        # memset is ISA-invalid for f32r tiles; stage ones in f32 and copy.
        onesf = pp.tile([128, 128], F32, tag="onesf", name="onesf")
        nc.vector.memset(onesf[:], 1.0)
        nc.vector.tensor_copy(ones_row[:], onesf[0:1, :])
        # ones column (col 64 of each 65-wide group) of V-tilde
        vt_ones = vt[:].rearrange("p (n c) -> p n c", c=65)[:, :, 64:65]
        nc.vector.tensor_copy(vt_ones, onesf[:, 0:64, None])

        # ================= Phase A: projections =================
        with (
            tc.tile_pool(name="xtp", bufs=2) as xp,
            tc.tile_pool(name="psA", bufs=2, space="PSUM") as psA,
        ):
            # b-inner order so the first 4 chunks cover (sc=0, t=0..3) of
            # every batch - lets attention start ~4x earlier
            for ck in [b * 4 + ssub for ssub in range(4) for b in range(4)]:
                b = ck // 4
                pair, row = b // 2, (b % 2) * 64
                xtile = [xp.tile([128, 512], r32, tag=f"xt{i}", name=f"xtile{i}") for i in range(4)]
                for i in range(4):
                    nc.sync.dma_start(
                        xtile[i][:],
                        xt_d[i * 128:(i + 1) * 128, ck * 512:(ck + 1) * 512].bitcast(r32),
                    )
                # Q^T | K^T (stacked 64+64) for this token chunk
                pqk = psA.tile([128, 512], F32, tag="pqk", name="pqk")
                for i in range(4):
                    nc.tensor.matmul(pqk[:], wqk[i][:], xtile[i][:],
                                     start=(i == 0), stop=(i == 3))
                scol = (ck % 4) * 512
                nc.scalar.activation(qt[pair][row:row + 64, scol:scol + 512],
                                     pqk[0:64, :], AF.Identity, bias=bqk[0:64, :])
                nc.scalar.activation(kt[pair][row:row + 64, scol:scol + 512],
                                     pqk[64:128, :], AF.Identity, bias=bqk[64:128, :])
                # V (natural layout) per 128-token subchunk, + ones-row bias fold
                for sub in range(4):
                    pv = psA.tile([128, DV], F32, tag="pv", name="pv")
                    for i in range(4):
                        nc.tensor.matmul(pv[:], xtile[i][:, sub * 128:(sub + 1) * 128],
                                         wv[i][:], start=(i == 0), stop=False)
                    nc.tensor.matmul(pv[:], ones_row[:], bv[:], start=False, stop=True)
                    tci = ck * 4 + sub  # global token-chunk index (b-major)
                    nc.vector.tensor_copy(vt[:, tci * 65:tci * 65 + 64], pv[:])

        # ================= Phase B: attention =================
        with (
            tc.tile_pool(name="wb", bufs=2) as wb,
            tc.tile_pool(name="psB", bufs=1, space="PSUM") as psB,
        ):
            # Software-pipelined over 64 global blocks g = sc*16 + t.
            # Per iteration: scores(g)+exp1(g) are emitted BEFORE the
            # DVE chain of g and exp2(g-1), so the ACT queue interleaves
            # exp1(g+1) ahead of exp2(g) and blocks overlap.
            NB = N_SC * N_TC
            pipe = {}  # g -> (e, p1) tiles

            def softmax_b(g):
                """scores(g) -> e(g) -> p1(g) tiles (no exp2 yet)."""
                sc, t = g // N_TC, g % N_TC
                scp = psB.tile([128, 4 * SC], F32, tag="scp", name="scp")
                for b in range(4):
                    pair, row = b // 2, (b % 2) * 64
                    nc.tensor.matmul(
                        scp[:, b * SC:(b + 1) * SC],
                        kt[pair][row:row + 64, t * TC:(t + 1) * TC],
                        qt[pair][row:row + 64, sc * SC:(sc + 1) * SC],
                        start=True, stop=True,
                    )
                # e = exp(scores/8) for all 4 b
                e = wb.tile([128, 4 * SC], BF16, tag="e", name="e")
                nc.scalar.activation(e[:], scp[:], AF.Exp, scale=0.125)
                # D = sum_b e ; r = 1/D (custom-DVE fast reciprocal keeps
                # ACT on the single exp table set - no table thrashing)
                t01 = wb.tile([128, 2 * SC], BF16, tag="t01", name="t01", bufs=1)
                nc.gpsimd.tensor_add(t01[:], e[:, 0:2 * SC], e[:, 2 * SC:4 * SC])
                dd = wb.tile([128, SC], F32, tag="dd", name="dd", bufs=1)
                nc.vector.tensor_add(dd[:], t01[:, 0:SC], t01[:, SC:2 * SC])
                rf = wb.tile([128, SC], F32, tag="rf", name="rf", bufs=1)
                nc.vector.reciprocal_approx_fast(rf[:], dd[:])
                # p1 = e * r, one TT with r broadcast along the 4-b free dim
                p1 = wb.tile([128, 4 * SC], BF16, tag="p1", name="p1")
                nc.vector.tensor_mul(
                    p1[:].rearrange("p (b s) -> p b s", b=4),
                    e[:].rearrange("p (b s) -> p b s", b=4),
                    rf[:, None, :].broadcast_to([128, 4, SC]),
                )
                pipe[g] = p1

            def exp2_and_z(g, zacc):
                """exp2(g) + Z accumulation (ones-col -> sum_t in row 64)."""
                t = g % N_TC
                p1 = pipe.pop(g)
                q = wb.tile([128, 4 * SC], BF16, tag="q", name="q")
                nc.scalar.activation(q[:], p1[:], AF.Exp)
                for b in range(4):
                    tci = b * 16 + t
                    nc.tensor.matmul(
                        zacc[:, b * SC:(b + 1) * SC],
                        vt[:, tci * 65:(tci + 1) * 65],
                        q[:, b * SC:(b + 1) * SC],
                        start=(t == 0), stop=(t == N_TC - 1),
                    )

            a2a_in_h = [dram.tile([N_CORES * 65, 512], BF16, tag=f"a2a_in{q}",
                                  name=f"a2a_in{q}") for q in range(2)]
            a2a_out_h = [dram.tile([N_CORES * 65, 512], BF16, tag=f"a2a_out{q}",
                                   name=f"a2a_out{q}") for q in range(2)]

            def emit_a2a(q):
                # chunk r = my head's Z^T cols for core r's half-q tokens:
                # tok = (r//2)*S + q*1024 + (r%2)*512 ... +512
                for r in range(N_CORES):
                    col = (r // 2) * S + q * 1024 + (r % 2) * 512
                    nc.sync.dma_start(a2a_in_h[q][r * 65:(r + 1) * 65, :],
                                      zt[:, col:col + 512])
                nc.gpsimd.collective_compute(
                    "AllToAll",
                    mybir.AluOpType.bypass,
                    replica_groups=[list(range(N_CORES))],
                    ins=[a2a_in_h[q][:].opt()],
                    outs=[a2a_out_h[q][:].opt()],
                )

            zacc = None
            for g in range(NB + 1):
                if g < NB:
                    if g % N_TC == 0:
                        prev_zacc = zacc
                        zacc = psB.tile([65, 4 * SC], F32, tag="zacc", name="zacc")
                    softmax_b(g)
                if g >= 1:
                    gz = g - 1
                    za = prev_zacc if (g % N_TC == 0 and g < NB) else zacc
                    exp2_and_z(gz, za)
                    if gz % N_TC == N_TC - 1:
                        # evacuate Z^T (+denominator row) to bf16
                        sc_done = gz // N_TC
                        for b in range(4):
                            col = b * S + sc_done * SC
                            nc.vector.tensor_copy(zt[:, col:col + SC],
                                                  za[:, b * SC:(b + 1) * SC])
                        if sc_done == 1:
                            emit_a2a(0)  # overlaps remaining attention
                        elif sc_done == 3:
                            emit_a2a(1)

        # ================= Phase C: output projection =================
        with (
            tc.tile_pool(name="wc", bufs=1) as wc,
            tc.tile_pool(name="oc", bufs=2) as oc,
            tc.tile_pool(name="psC", bufs=2, space="PSUM") as psC,
        ):
            HT = 512  # tokens per half
            for q in range(2):
                zc = wc.tile([65, N_CORES * HT], BF16, tag="zc", name="zc", bufs=2)
                for j in range(N_CORES):
                    nc.sync.dma_start(zc[:, j * HT:(j + 1) * HT],
                                      a2a_out_h[q][j * 65:(j + 1) * 65, :])
                # r2 = exp(-ln(denom)) per (head, token); bcast to 64 rows
                lden = wc.tile([1, N_CORES * HT], F32, tag="lden", name="lden", bufs=2)
                nc.scalar.activation(lden[:], zc[64:65, :], AF.Ln)
                rden = wc.tile([1, N_CORES * HT], BF16, tag="rden", name="rden", bufs=2)
                nc.scalar.activation(rden[:], lden[:], AF.Exp, scale=-1.0)
                rb = wc.tile([64, N_CORES * HT], BF16, tag="rb", name="rb", bufs=2)
                nc.gpsimd.partition_broadcast(rb[:], rden[:])
                # normalized Zc^T in hd-major pairs: tile i = heads 2i, 2i+1
                zcn = [wc.tile([128, HT], r32, tag=f"zcn{i}", name=f"zcn{i}", bufs=2)
                       for i in range(4)]
                for j in range(N_CORES):
                    nc.vector.tensor_mul(
                        zcn[j // 2][(j % 2) * 64:(j % 2) * 64 + 64, :],
                        zc[0:64, j * HT:(j + 1) * HT],
                        rb[:, j * HT:(j + 1) * HT],
                    )
                for m in range(HT // 128):
                    po = psC.tile([128, D], F32, tag="po", name="po")
                    for i in range(4):
                        nc.tensor.matmul(po[:], zcn[i][:, m * 128:(m + 1) * 128],
                                         wo[i][:], start=(i == 0), stop=False)
                    nc.tensor.matmul(po[:], ones_row[:], bo[:], start=False,
                                     stop=True)
                    ot = oc.tile([128, D], F32, tag="ot", name="ot")
                    nc.vector.tensor_copy(ot[:], po[:])
                    row = q * HT + m * 128
                    nc.sync.dma_start(out_d[row:row + 128, :], ot[:])

    nc.compile()
    return nc


_NC_CACHE = None


def _get_nc():
    global _NC_CACHE
    if _NC_CACHE is None:
        _NC_CACHE = build_kernel()
    return _NC_CACHE


def kernel(X, WQ, bQ, WK, bK, WV, bV, WO, bO, _trace=False, _trace_kwargs=None):
    """Full inputs in, full output out. Shards internally across 8 cores."""
    X = np.asarray(X, dtype=np.float32)
    # [S,B,D] -> XT [D, NTOK] with b-major tokens (tok = b*S + s)
    xt = np.ascontiguousarray(X.transpose(2, 1, 0).reshape(D, NTOK))
    in_maps = []
    for h in range(N_CORES):
        wqk = np.ascontiguousarray(
            np.concatenate([WQ[h], WK[h]], axis=1), dtype=np.float32)
        bqk = np.ascontiguousarray(
            np.concatenate([bQ[h], bK[h]])[:, None], dtype=np.float32)
        in_maps.append({
            "xt": xt,
            "wqk": wqk,
            "bqk": bqk,
            "wv": np.ascontiguousarray(WV[h], dtype=np.float32),
            "bv": np.ascontiguousarray(bV[h][None, :], dtype=np.float32),
            "wo": np.ascontiguousarray(WO, dtype=np.float32),
            "bo": np.ascontiguousarray(bO[None, :], dtype=np.float32),
        })
    nc = _get_nc()
    res = run_bass_kernel_spmd(
        nc, in_maps, core_ids=list(range(N_CORES)),
        trace=_trace, **(_trace_kwargs or {}),
    )
    # core c rows: [0:512] = tokens (c//2)*S + (c%2)*512 .. ; [512:1024] same + 1024
    fullb = np.empty((B, S, D), dtype=np.float32)
    for c in range(N_CORES):
        oc = res.results[c]["out"]
        b, off = c // 2, (c % 2) * 512
        fullb[b, off:off + 512] = oc[0:512]
        fullb[b, 1024 + off:1024 + off + 512] = oc[512:1024]
    full = fullb.transpose(1, 0, 2)
    if _trace:
        return np.ascontiguousarray(full), res
    return np.ascontiguousarray(full)


# revision 15
# speedup vs baseline: 1.3476x; 1.0296x over previous
"""Trainium2 Bass kernel for nn_Encoder_78795470012907.

Encoder layer: per-head Q/K/V projections, scores = QK^T/sqrt(dk),
double softmax (over batch axis, then over key axis), Z = pV, concat
heads, output projection. S=2048, B=4, D=512, H=8, dk=dv=64.

Sharding: head-parallel over 8 cores (core h owns head h) for the
attention; AllToAll re-shards by token for the output projection, so
each core emits a disjoint 1024-token slice of the output (host just
concatenates).

Layout notes (per core):
 - tokens are b-major: tok = b*2048 + s.
 - X is fed pre-transposed from host as XT [D, NTOK] (pure layout prep).
 - projections produce Q^T/K^T [dk, tok] stacked in b-pairs so the
   scores matmuls row-pack two batches into the 128x128 PE array.
 - scores are computed transposed ([t, s] tiles); the softmax over the
   key axis t rides the Z matmul via a ones-column appended to V
   (row 64 of the Z psum accumulates sum_t exp).
 - softmax over b: e=exp(s/8) -> D=sum_b e -> r=exp(-ln D) -> p1=e*r.
   ln+exp coexist in one ACT table set (no table thrashing);
   Reciprocal would force a table switch per tile.
"""

from contextlib import ExitStack

import numpy as np

import concourse.bass as bass
import concourse.tile as tile
from concourse import bacc, mybir
from concourse.bass_utils import run_bass_kernel_spmd

S, B, D = 2048, 4, 512
H, DK, DV = 8, 64, 64
N_CORES = 8
NTOK = S * B          # 8192 tokens, b-major
TOKC = NTOK // N_CORES  # 1024 tokens per core for the output slice
SC = 512              # s-chunk (columns of a scores^T tile)
TC = 128              # t-chunk (partitions of a scores^T tile)
N_SC = S // SC        # 4
N_TC = S // TC        # 16

F32 = mybir.dt.float32
F32R = mybir.dt.float32r
BF16 = mybir.dt.bfloat16
AF = mybir.ActivationFunctionType


def build_kernel():
    nc = bacc.Bacc(num_devices=N_CORES)

    xt_d = nc.dram_tensor("xt", [D, NTOK], F32, kind="ExternalInput")
    wqk_d = nc.dram_tensor("wqk", [D, 128], F32, kind="ExternalInput")
    bqk_d = nc.dram_tensor("bqk", [128, 1], F32, kind="ExternalInput")
    wv_d = nc.dram_tensor("wv", [D, DV], F32, kind="ExternalInput")
    bv_d = nc.dram_tensor("bv", [1, DV], F32, kind="ExternalInput")
    wo_d = nc.dram_tensor("wo", [D, D], F32, kind="ExternalInput")
    bo_d = nc.dram_tensor("bo", [1, D], F32, kind="ExternalInput")
    out_d = nc.dram_tensor("out", [TOKC, D], F32, kind="ExternalOutput")

    r32 = mybir.dt.float32r

    with tile.TileContext(nc) as tc, ExitStack() as ctx:
        pp = ctx.enter_context(tc.tile_pool(name="persist", bufs=1))
        dram = ctx.enter_context(tc.tile_pool(name="dram", bufs=1, space="DRAM"))

        # ---- persistent SBUF ----
        # Q^T/K^T in b-pairs: rows 0:64 = batch 2p, rows 64:128 = batch 2p+1
        qt = [pp.tile([128, S], BF16, tag=f"qt{p}", name=f"qt{p}") for p in range(2)]
        kt = [pp.tile([128, S], BF16, tag=f"kt{p}", name=f"kt{p}") for p in range(2)]
        # V-tilde: 64 token-chunks of [128 tok, 65] (col 64 = ones)
        vt = pp.tile([128, 64 * 65], BF16, tag="vt", name="vt")
        # Z^T (unnormalized) + denom row: [65, NTOK]
        zt = pp.tile([65, NTOK], BF16, tag="zt", name="zt")

        # weights
        wqk = [pp.tile([128, 128], r32, tag=f"wqk{i}", name=f"wqk{i}") for i in range(4)]
        wv = [pp.tile([128, DV], r32, tag=f"wv{i}", name=f"wv{i}") for i in range(4)]
        wo = [pp.tile([128, D], r32, tag=f"wo{i}", name=f"wo{i}") for i in range(4)]
        bqk = pp.tile([128, 1], F32, tag="bqk", name="bqk")
        bv = pp.tile([1, DV], r32, tag="bv", name="bv")
        bo = pp.tile([1, D], r32, tag="bo", name="bo")
        ones_row = pp.tile([1, 128], r32, tag="ones_row", name="ones_row")

        for i in range(4):
            nc.sync.dma_start(wqk[i][:], wqk_d[i * 128:(i + 1) * 128, :].bitcast(r32))
            nc.sync.dma_start(wv[i][:], wv_d[i * 128:(i + 1) * 128, :].bitcast(r32))
            nc.sync.dma_start(wo[i][:], wo_d[i * 128:(i + 1) * 128, :].bitcast(r32))
        nc.sync.dma_start(bqk[:], bqk_d[:])
        nc.sync.dma_start(bv[:], bv_d[:].bitcast(r32))
        nc.sync.dma_start(bo[:], bo_d[:].bitcast(r32))
        # memset is ISA-invalid for f32r tiles; stage ones in f32 and copy.
        onesf = pp.tile([128, 128], F32, tag="onesf", name="onesf")
        nc.vector.memset(onesf[:], 1.0)
        nc.vector.tensor_copy(ones_row[:], onesf[0:1, :])
        # ones column (col 64 of each 65-wide group) of V-tilde
        vt_ones = vt[:].rearrange("p (n c) -> p n c", c=65)[:, :, 64:65]
        nc.vector.tensor_copy(vt_ones, onesf[:, 0:64, None])

        # ================= Phase A: projections =================
        with (
            tc.tile_pool(name="xtp", bufs=2) as xp,
            tc.tile_pool(name="psA", bufs=2, space="PSUM") as psA,
        ):
            # b-inner order so the first 4 chunks cover (sc=0, t=0..3) of
            # every batch - lets attention start ~4x earlier
            for ck in [b * 4 + ssub for ssub in range(4) for b in range(4)]:
                b = ck // 4
                pair, row = b // 2, (b % 2) * 64
                xtile = [xp.tile([128, 512], r32, tag=f"xt{i}", name=f"xtile{i}") for i in range(4)]
                for i in range(4):
                    nc.sync.dma_start(
                        xtile[i][:],
                        xt_d[i * 128:(i + 1) * 128, ck * 512:(ck + 1) * 512].bitcast(r32),
                    )
                # Q^T | K^T (stacked 64+64) for this token chunk
                pqk = psA.tile([128, 512], F32, tag="pqk", name="pqk")
                for i in range(4):
                    nc.tensor.matmul(pqk[:], wqk[i][:], xtile[i][:],
                                     start=(i == 0), stop=(i == 3))
                scol = (ck % 4) * 512
                nc.scalar.activation(qt[pair][row:row + 64, scol:scol + 512],
                                     pqk[0:64, :], AF.Identity, bias=bqk[0:64, :])
                nc.scalar.activation(kt[pair][row:row + 64, scol:scol + 512],
                                     pqk[64:128, :], AF.Identity, bias=bqk[64:128, :])
                # V (natural layout) per 128-token subchunk, + ones-row bias fold
                for sub in range(4):
                    pv = psA.tile([128, DV], F32, tag="pv", name="pv")
                    for i in range(4):
                        nc.tensor.matmul(pv[:], xtile[i][:, sub * 128:(sub + 1) * 128],
                                         wv[i][:], start=(i == 0), stop=False)
                    nc.tensor.matmul(pv[:], ones_row[:], bv[:], start=False, stop=True)
                    tci = ck * 4 + sub  # global token-chunk index (b-major)
                    nc.vector.tensor_copy(vt[:, tci * 65:tci * 65 + 64], pv[:])

        # ================= Phase B: attention =================
        with (
            tc.tile_pool(name="wb", bufs=2) as wb,
            tc.tile_pool(name="psB", bufs=1, space="PSUM") as psB,
        ):
            # Software-pipelined over 64 global blocks g = sc*16 + t.
            # Per iteration: scores(g)+exp1(g) are emitted BEFORE the
            # DVE chain of g and exp2(g-1), so the ACT queue interleaves
            # exp1(g+1) ahead of exp2(g) and blocks overlap.
            NB = N_SC * N_TC
            pipe = {}  # g -> (e, p1) tiles

            def softmax_b(g):
                """scores(g) -> e(g) -> p1(g) tiles (no exp2 yet)."""
                sc, t = g // N_TC, g % N_TC
                scp = psB.tile([128, 4 * SC], F32, tag="scp", name="scp")
                for b in range(4):
                    pair, row = b // 2, (b % 2) * 64
                    nc.tensor.matmul(
                        scp[:, b * SC:(b + 1) * SC],
                        kt[pair][row:row + 64, t * TC:(t + 1) * TC],
                        qt[pair][row:row + 64, sc * SC:(sc + 1) * SC],
                        start=True, stop=True,
                    )
                # e = exp(scores/8) for all 4 b
                e = wb.tile([128, 4 * SC], BF16, tag="e", name="e", bufs=3)
                nc.scalar.activation(e[:], scp[:], AF.Exp, scale=0.125)
                # D = sum_b e ; r = 1/D (custom-DVE fast reciprocal keeps
                # ACT on the single exp table set - no table thrashing)
                t01 = wb.tile([128, 2 * SC], BF16, tag="t01", name="t01", bufs=2)
                nc.gpsimd.tensor_add(t01[:], e[:, 0:2 * SC], e[:, 2 * SC:4 * SC])
                dd = wb.tile([128, SC], F32, tag="dd", name="dd", bufs=2)
                nc.vector.tensor_add(dd[:], t01[:, 0:SC], t01[:, SC:2 * SC])
                rf = wb.tile([128, SC], F32, tag="rf", name="rf", bufs=2)
                nc.vector.reciprocal_approx_fast(rf[:], dd[:])
                # p1 = e * r, one TT with r broadcast along the 4-b free dim
                p1 = wb.tile([128, 4 * SC], BF16, tag="p1", name="p1", bufs=3)
                nc.vector.tensor_mul(
                    p1[:].rearrange("p (b s) -> p b s", b=4),
                    e[:].rearrange("p (b s) -> p b s", b=4),
                    rf[:, None, :].broadcast_to([128, 4, SC]),
                )
                pipe[g] = p1

            def exp2_and_z(g, zacc):
                """exp2(g) + Z accumulation (ones-col -> sum_t in row 64)."""
                t = g % N_TC
                p1 = pipe.pop(g)
                q = wb.tile([128, 4 * SC], BF16, tag="q", name="q", bufs=3)
                nc.scalar.activation(q[:], p1[:], AF.Exp)
                for b in range(4):
                    tci = b * 16 + t
                    nc.tensor.matmul(
                        zacc[:, b * SC:(b + 1) * SC],
                        vt[:, tci * 65:(tci + 1) * 65],
                        q[:, b * SC:(b + 1) * SC],
                        start=(t == 0), stop=(t == N_TC - 1),
                    )

            a2a_in_h = [dram.tile([N_CORES * 65, 512], BF16, tag=f"a2a_in{q}",
                                  name=f"a2a_in{q}") for q in range(2)]
            a2a_out_h = [dram.tile([N_CORES * 65, 512], BF16, tag=f"a2a_out{q}",
                                   name=f"a2a_out{q}") for q in range(2)]

            def emit_a2a(q):
                # chunk r = my head's Z^T cols for core r's half-q tokens:
                # tok = (r//2)*S + q*1024 + (r%2)*512 ... +512
                for r in range(N_CORES):
                    col = (r // 2) * S + q * 1024 + (r % 2) * 512
                    nc.sync.dma_start(a2a_in_h[q][r * 65:(r + 1) * 65, :],
                                      zt[:, col:col + 512])
                nc.gpsimd.collective_compute(
                    "AllToAll",
                    mybir.AluOpType.bypass,
                    replica_groups=[list(range(N_CORES))],
                    ins=[a2a_in_h[q][:].opt()],
                    outs=[a2a_out_h[q][:].opt()],
                )

            zaccs = {}
            for g in range(NB + 2):
                if g < NB:
                    if g % N_TC == 0:
                        zaccs[g // N_TC] = psB.tile([65, 4 * SC], F32,
                                                    tag="zacc", name="zacc")
                    softmax_b(g)
                if g >= 2:
                    gz = g - 2
                    za = zaccs[gz // N_TC]
                    exp2_and_z(gz, za)
                    if gz % N_TC == N_TC - 1:
                        # evacuate Z^T (+denominator row) to bf16
                        sc_done = gz // N_TC
                        for b in range(4):
                            col = b * S + sc_done * SC
                            nc.vector.tensor_copy(zt[:, col:col + SC],
                                                  za[:, b * SC:(b + 1) * SC])
                        if sc_done == 1:
                            emit_a2a(0)  # overlaps remaining attention
                        elif sc_done == 3:
                            emit_a2a(1)

        # ================= Phase C: output projection =================
        with (
            tc.tile_pool(name="wc", bufs=1) as wc,
            tc.tile_pool(name="oc", bufs=2) as oc,
            tc.tile_pool(name="psC", bufs=2, space="PSUM") as psC,
        ):
            HT = 512  # tokens per half
            for q in range(2):
                zc = wc.tile([65, N_CORES * HT], BF16, tag="zc", name="zc", bufs=2)
                for j in range(N_CORES):
                    nc.sync.dma_start(zc[:, j * HT:(j + 1) * HT],
                                      a2a_out_h[q][j * 65:(j + 1) * 65, :])
                # r2 = exp(-ln(denom)) per (head, token); bcast to 64 rows
                lden = wc.tile([1, N_CORES * HT], F32, tag="lden", name="lden", bufs=2)
                nc.scalar.activation(lden[:], zc[64:65, :], AF.Ln)
                rden = wc.tile([1, N_CORES * HT], BF16, tag="rden", name="rden", bufs=2)
                nc.scalar.activation(rden[:], lden[:], AF.Exp, scale=-1.0)
                rb = wc.tile([64, N_CORES * HT], BF16, tag="rb", name="rb", bufs=2)
                nc.gpsimd.partition_broadcast(rb[:], rden[:])
                # normalized Zc^T in hd-major pairs: tile i = heads 2i, 2i+1
                zcn = [wc.tile([128, HT], r32, tag=f"zcn{i}", name=f"zcn{i}", bufs=2)
                       for i in range(4)]
                for j in range(N_CORES):
                    nc.vector.tensor_mul(
                        zcn[j // 2][(j % 2) * 64:(j % 2) * 64 + 64, :],
                        zc[0:64, j * HT:(j + 1) * HT],
                        rb[:, j * HT:(j + 1) * HT],
                    )
                for m in range(HT // 128):
                    po = psC.tile([128, D], F32, tag="po", name="po")
                    for i in range(4):
                        nc.tensor.matmul(po[:], zcn[i][:, m * 128:(m + 1) * 128],
                                         wo[i][:], start=(i == 0), stop=False)
                    nc.tensor.matmul(po[:], ones_row[:], bo[:], start=False,
                                     stop=True)
                    ot = oc.tile([128, D], F32, tag="ot", name="ot")
                    nc.vector.tensor_copy(ot[:], po[:])
                    row = q * HT + m * 128
                    nc.sync.dma_start(out_d[row:row + 128, :], ot[:])

    nc.compile()
    return nc


_NC_CACHE = None


def _get_nc():
    global _NC_CACHE
    if _NC_CACHE is None:
        _NC_CACHE = build_kernel()
    return _NC_CACHE


def kernel(X, WQ, bQ, WK, bK, WV, bV, WO, bO, _trace=False, _trace_kwargs=None):
    """Full inputs in, full output out. Shards internally across 8 cores."""
    X = np.asarray(X, dtype=np.float32)
    # [S,B,D] -> XT [D, NTOK] with b-major tokens (tok = b*S + s)
    xt = np.ascontiguousarray(X.transpose(2, 1, 0).reshape(D, NTOK))
    in_maps = []
    for h in range(N_CORES):
        wqk = np.ascontiguousarray(
            np.concatenate([WQ[h], WK[h]], axis=1), dtype=np.float32)
        bqk = np.ascontiguousarray(
            np.concatenate([bQ[h], bK[h]])[:, None], dtype=np.float32)
        in_maps.append({
            "xt": xt,
            "wqk": wqk,
            "bqk": bqk,
            "wv": np.ascontiguousarray(WV[h], dtype=np.float32),
            "bv": np.ascontiguousarray(bV[h][None, :], dtype=np.float32),
            "wo": np.ascontiguousarray(WO, dtype=np.float32),
            "bo": np.ascontiguousarray(bO[None, :], dtype=np.float32),
        })
    nc = _get_nc()
    res = run_bass_kernel_spmd(
        nc, in_maps, core_ids=list(range(N_CORES)),
        trace=_trace, **(_trace_kwargs or {}),
    )
    # core c rows: [0:512] = tokens (c//2)*S + (c%2)*512 .. ; [512:1024] same + 1024
    fullb = np.empty((B, S, D), dtype=np.float32)
    for c in range(N_CORES):
        oc = res.results[c]["out"]
        b, off = c // 2, (c % 2) * 512
        fullb[b, off:off + 512] = oc[0:512]
        fullb[b, 1024 + off:1024 + off + 512] = oc[512:1024]
    full = fullb.transpose(1, 0, 2)
    if _trace:
        return np.ascontiguousarray(full), res
    return np.ascontiguousarray(full)


# revision 16
# speedup vs baseline: 1.6200x; 1.2021x over previous
"""Trainium2 Bass kernel for nn_Encoder_78795470012907.

Encoder layer: per-head Q/K/V projections, scores = QK^T/sqrt(dk),
double softmax (over batch axis, then over key axis), Z = pV, concat
heads, output projection. S=2048, B=4, D=512, H=8, dk=dv=64.

Sharding: head-parallel over 8 cores (core h owns head h) for the
attention; AllToAll re-shards by token for the output projection, so
each core emits a disjoint 1024-token slice of the output (host just
concatenates).

Layout notes (per core):
 - tokens are b-major: tok = b*2048 + s.
 - X is fed pre-transposed from host as XT [D, NTOK] (pure layout prep).
 - projections produce Q^T/K^T [dk, tok] stacked in b-pairs so the
   scores matmuls row-pack two batches into the 128x128 PE array.
 - scores are computed transposed ([t, s] tiles); the softmax over the
   key axis t rides the Z matmul via a ones-column appended to V
   (row 64 of the Z psum accumulates sum_t exp).
 - softmax over b: e=exp(s/8) -> D=sum_b e -> r=exp(-ln D) -> p1=e*r.
   ln+exp coexist in one ACT table set (no table thrashing);
   Reciprocal would force a table switch per tile.
"""

from contextlib import ExitStack

import numpy as np

import concourse.bass as bass
import concourse.tile as tile
from concourse import bacc, mybir
from concourse.bass_utils import run_bass_kernel_spmd

S, B, D = 2048, 4, 512
H, DK, DV = 8, 64, 64
N_CORES = 8
NTOK = S * B          # 8192 tokens, b-major
TOKC = NTOK // N_CORES  # 1024 tokens per core for the output slice
SC = 512              # s-chunk (columns of a scores^T tile)
TC = 128              # t-chunk (partitions of a scores^T tile)
N_SC = S // SC        # 4
N_TC = S // TC        # 16

F32 = mybir.dt.float32
F32R = mybir.dt.float32r
BF16 = mybir.dt.bfloat16
AF = mybir.ActivationFunctionType


def build_kernel():
    nc = bacc.Bacc(num_devices=N_CORES)

    xt_d = nc.dram_tensor("xt", [D, NTOK], F32, kind="ExternalInput")
    wqk_d = nc.dram_tensor("wqk", [D, 128], F32, kind="ExternalInput")
    bqk_d = nc.dram_tensor("bqk", [128, 1], F32, kind="ExternalInput")
    wv_d = nc.dram_tensor("wv", [D, DV], F32, kind="ExternalInput")
    bv_d = nc.dram_tensor("bv", [1, DV], F32, kind="ExternalInput")
    wo_d = nc.dram_tensor("wo", [D, D], F32, kind="ExternalInput")
    bo_d = nc.dram_tensor("bo", [1, D], F32, kind="ExternalInput")
    out_d = nc.dram_tensor("out", [TOKC, D], F32, kind="ExternalOutput")

    r32 = mybir.dt.float32r

    with tile.TileContext(nc) as tc, ExitStack() as ctx:
        pp = ctx.enter_context(tc.tile_pool(name="persist", bufs=1))
        dram = ctx.enter_context(tc.tile_pool(name="dram", bufs=1, space="DRAM"))

        # ---- persistent SBUF ----
        # Q^T/K^T in b-pairs: rows 0:64 = batch 2p, rows 64:128 = batch 2p+1
        qt = [pp.tile([128, S], BF16, tag=f"qt{p}", name=f"qt{p}") for p in range(2)]
        kt = [pp.tile([128, S], BF16, tag=f"kt{p}", name=f"kt{p}") for p in range(2)]
        # V-tilde: 64 token-chunks of [128 tok, 65] (col 64 = ones)
        vt = pp.tile([128, 64 * 65], BF16, tag="vt", name="vt")
        # Z^T (unnormalized) + denom row: [65, NTOK]
        zt = pp.tile([65, NTOK], BF16, tag="zt", name="zt")

        # weights
        wqk = [pp.tile([128, 128], r32, tag=f"wqk{i}", name=f"wqk{i}") for i in range(4)]
        wv = [pp.tile([128, DV], r32, tag=f"wv{i}", name=f"wv{i}") for i in range(4)]
        wo = [pp.tile([128, D], r32, tag=f"wo{i}", name=f"wo{i}") for i in range(4)]
        bqk = pp.tile([128, 1], F32, tag="bqk", name="bqk")
        bv = pp.tile([1, DV], r32, tag="bv", name="bv")
        bo = pp.tile([1, D], r32, tag="bo", name="bo")
        ones_row = pp.tile([1, 128], r32, tag="ones_row", name="ones_row")

        for i in range(4):
            nc.sync.dma_start(wqk[i][:], wqk_d[i * 128:(i + 1) * 128, :].bitcast(r32))
            nc.sync.dma_start(wv[i][:], wv_d[i * 128:(i + 1) * 128, :].bitcast(r32))
            nc.sync.dma_start(wo[i][:], wo_d[i * 128:(i + 1) * 128, :].bitcast(r32))
        nc.sync.dma_start(bqk[:], bqk_d[:])
        nc.sync.dma_start(bv[:], bv_d[:].bitcast(r32))
        nc.sync.dma_start(bo[:], bo_d[:].bitcast(r32))
        # memset is ISA-invalid for f32r tiles; stage ones in f32 and copy.
        onesf = pp.tile([128, 128], F32, tag="onesf", name="onesf")
        nc.vector.memset(onesf[:], 1.0)
        nc.vector.tensor_copy(ones_row[:], onesf[0:1, :])
        # ones column (col 64 of each 65-wide group) of V-tilde
        vt_ones = vt[:].rearrange("p (n c) -> p n c", c=65)[:, :, 64:65]
        nc.vector.tensor_copy(vt_ones, onesf[:, 0:64, None])

        # ================= Phase A: projections =================
        with (
            tc.tile_pool(name="xtp", bufs=2) as xp,
            tc.tile_pool(name="psA", bufs=2, space="PSUM") as psA,
        ):
            # b-inner order so the first 4 chunks cover (sc=0, t=0..3) of
            # every batch - lets attention start ~4x earlier
            for ck in [b * 4 + ssub for ssub in range(4) for b in range(4)]:
                b = ck // 4
                pair, row = b // 2, (b % 2) * 64
                xtile = [xp.tile([128, 512], r32, tag=f"xt{i}", name=f"xtile{i}") for i in range(4)]
                for i in range(4):
                    nc.sync.dma_start(
                        xtile[i][:],
                        xt_d[i * 128:(i + 1) * 128, ck * 512:(ck + 1) * 512].bitcast(r32),
                    )
                # Q^T | K^T (stacked 64+64) for this token chunk
                pqk = psA.tile([128, 512], F32, tag="pqk", name="pqk")
                for i in range(4):
                    nc.tensor.matmul(pqk[:], wqk[i][:], xtile[i][:],
                                     start=(i == 0), stop=(i == 3))
                scol = (ck % 4) * 512
                nc.scalar.activation(qt[pair][row:row + 64, scol:scol + 512],
                                     pqk[0:64, :], AF.Identity, bias=bqk[0:64, :])
                nc.scalar.activation(kt[pair][row:row + 64, scol:scol + 512],
                                     pqk[64:128, :], AF.Identity, bias=bqk[64:128, :])
                # V (natural layout) per 128-token subchunk, + ones-row bias fold
                for sub in range(4):
                    pv = psA.tile([128, DV], F32, tag="pv", name="pv")
                    for i in range(4):
                        nc.tensor.matmul(pv[:], xtile[i][:, sub * 128:(sub + 1) * 128],
                                         wv[i][:], start=(i == 0), stop=False)
                    nc.tensor.matmul(pv[:], ones_row[:], bv[:], start=False, stop=True)
                    tci = ck * 4 + sub  # global token-chunk index (b-major)
                    nc.vector.tensor_copy(vt[:, tci * 65:tci * 65 + 64], pv[:])

        # ================= Phase B: attention =================
        with (
            tc.tile_pool(name="wb", bufs=2) as wb,
            tc.tile_pool(name="psB", bufs=1, space="PSUM") as psB,
        ):
            # Software-pipelined over 64 global blocks g = sc*16 + t.
            # Per iteration: scores(g)+exp1(g) are emitted BEFORE the
            # DVE chain of g and exp2(g-1), so the ACT queue interleaves
            # exp1(g+1) ahead of exp2(g) and blocks overlap.
            NB = N_SC * N_TC
            pipe = {}  # g -> (e, p1) tiles

            def softmax_b(g):
                """scores(g) -> e(g) -> p1(g) tiles (no exp2 yet)."""
                sc, t = g // N_TC, g % N_TC
                scp = psB.tile([128, 4 * SC], F32, tag="scp", name="scp")
                for b in range(4):
                    pair, row = b // 2, (b % 2) * 64
                    nc.tensor.matmul(
                        scp[:, b * SC:(b + 1) * SC],
                        kt[pair][row:row + 64, t * TC:(t + 1) * TC],
                        qt[pair][row:row + 64, sc * SC:(sc + 1) * SC],
                        start=True, stop=True,
                    )
                # e = exp(scores/8) for all 4 b
                e = wb.tile([128, 4 * SC], BF16, tag="e", name="e", bufs=3)
                nc.scalar.activation(e[:], scp[:], AF.Exp, scale=0.125)
                # D = sum_b e ; r = 1/D (custom-DVE fast reciprocal keeps
                # ACT on the single exp table set - no table thrashing)
                t01 = wb.tile([128, 2 * SC], BF16, tag="t01", name="t01", bufs=2)
                nc.vector.tensor_add(t01[:], e[:, 0:2 * SC], e[:, 2 * SC:4 * SC])
                dd = wb.tile([128, SC], BF16, tag="dd", name="dd", bufs=2)
                nc.vector.tensor_add(dd[:], t01[:, 0:SC], t01[:, SC:2 * SC])
                ddf = wb.tile([128, SC], F32, tag="ddf", name="ddf", bufs=2)
                nc.vector.tensor_copy(ddf[:], dd[:])
                rf = wb.tile([128, SC], F32, tag="rf", name="rf", bufs=2)
                nc.vector.reciprocal_approx_fast(rf[:], ddf[:])
                # p1 = e * r, one TT with r broadcast along the 4-b free dim
                p1 = wb.tile([128, 4 * SC], BF16, tag="p1", name="p1", bufs=3)
                nc.vector.tensor_mul(
                    p1[:].rearrange("p (b s) -> p b s", b=4),
                    e[:].rearrange("p (b s) -> p b s", b=4),
                    rf[:, None, :].broadcast_to([128, 4, SC]),
                )
                pipe[g] = p1

            def exp2_and_z(g, zacc):
                """exp2(g) + Z accumulation (ones-col -> sum_t in row 64)."""
                t = g % N_TC
                p1 = pipe.pop(g)
                q = wb.tile([128, 4 * SC], BF16, tag="q", name="q", bufs=3)
                nc.scalar.activation(q[:], p1[:], AF.Exp)
                for b in range(4):
                    tci = b * 16 + t
                    nc.tensor.matmul(
                        zacc[:, b * SC:(b + 1) * SC],
                        vt[:, tci * 65:(tci + 1) * 65],
                        q[:, b * SC:(b + 1) * SC],
                        start=(t == 0), stop=(t == N_TC - 1),
                    )

            a2a_in_h = [dram.tile([N_CORES * 65, 512], BF16, tag=f"a2a_in{q}",
                                  name=f"a2a_in{q}") for q in range(2)]
            a2a_out_h = [dram.tile([N_CORES * 65, 512], BF16, tag=f"a2a_out{q}",
                                   name=f"a2a_out{q}") for q in range(2)]

            def emit_a2a(q):
                # chunk r = my head's Z^T cols for core r's half-q tokens:
                # tok = (r//2)*S + q*1024 + (r%2)*512 ... +512
                for r in range(N_CORES):
                    col = (r // 2) * S + q * 1024 + (r % 2) * 512
                    nc.sync.dma_start(a2a_in_h[q][r * 65:(r + 1) * 65, :],
                                      zt[:, col:col + 512])
                nc.gpsimd.collective_compute(
                    "AllToAll",
                    mybir.AluOpType.bypass,
                    replica_groups=[list(range(N_CORES))],
                    ins=[a2a_in_h[q][:].opt()],
                    outs=[a2a_out_h[q][:].opt()],
                )

            zaccs = {}
            for g in range(NB + 2):
                if g < NB:
                    if g % N_TC == 0:
                        zaccs[g // N_TC] = psB.tile([65, 4 * SC], F32,
                                                    tag="zacc", name="zacc")
                    softmax_b(g)
                if g >= 2:
                    gz = g - 2
                    za = zaccs[gz // N_TC]
                    exp2_and_z(gz, za)
                    if gz % N_TC == N_TC - 1:
                        # evacuate Z^T (+denominator row) to bf16
                        sc_done = gz // N_TC
                        for b in range(4):
                            col = b * S + sc_done * SC
                            nc.vector.tensor_copy(zt[:, col:col + SC],
                                                  za[:, b * SC:(b + 1) * SC])
                        if sc_done == 1:
                            emit_a2a(0)  # overlaps remaining attention
                        elif sc_done == 3:
                            emit_a2a(1)

        # ================= Phase C: output projection =================
        with (
            tc.tile_pool(name="wc", bufs=1) as wc,
            tc.tile_pool(name="oc", bufs=2) as oc,
            tc.tile_pool(name="psC", bufs=2, space="PSUM") as psC,
        ):
            HT = 512  # tokens per half
            for q in range(2):
                zc = wc.tile([65, N_CORES * HT], BF16, tag="zc", name="zc", bufs=2)
                for j in range(N_CORES):
                    nc.sync.dma_start(zc[:, j * HT:(j + 1) * HT],
                                      a2a_out_h[q][j * 65:(j + 1) * 65, :])
                # r2 = exp(-ln(denom)) per (head, token); bcast to 64 rows
                lden = wc.tile([1, N_CORES * HT], F32, tag="lden", name="lden", bufs=2)
                nc.scalar.activation(lden[:], zc[64:65, :], AF.Ln)
                rden = wc.tile([1, N_CORES * HT], BF16, tag="rden", name="rden", bufs=2)
                nc.scalar.activation(rden[:], lden[:], AF.Exp, scale=-1.0)
                rb = wc.tile([64, N_CORES * HT], BF16, tag="rb", name="rb", bufs=2)
                nc.gpsimd.partition_broadcast(rb[:], rden[:])
                # normalized Zc^T in hd-major pairs: tile i = heads 2i, 2i+1
                zcn = [wc.tile([128, HT], r32, tag=f"zcn{i}", name=f"zcn{i}", bufs=2)
                       for i in range(4)]
                for j in range(N_CORES):
                    nc.vector.tensor_mul(
                        zcn[j // 2][(j % 2) * 64:(j % 2) * 64 + 64, :],
                        zc[0:64, j * HT:(j + 1) * HT],
                        rb[:, j * HT:(j + 1) * HT],
                    )
                for m in range(HT // 128):
                    po = psC.tile([128, D], F32, tag="po", name="po")
                    for i in range(4):
                        nc.tensor.matmul(po[:], zcn[i][:, m * 128:(m + 1) * 128],
                                         wo[i][:], start=(i == 0), stop=False)
                    nc.tensor.matmul(po[:], ones_row[:], bo[:], start=False,
                                     stop=True)
                    ot = oc.tile([128, D], F32, tag="ot", name="ot")
                    nc.vector.tensor_copy(ot[:], po[:])
                    row = q * HT + m * 128
                    nc.sync.dma_start(out_d[row:row + 128, :], ot[:])

    nc.compile()
    return nc


_NC_CACHE = None


def _get_nc():
    global _NC_CACHE
    if _NC_CACHE is None:
        _NC_CACHE = build_kernel()
    return _NC_CACHE


def kernel(X, WQ, bQ, WK, bK, WV, bV, WO, bO, _trace=False, _trace_kwargs=None):
    """Full inputs in, full output out. Shards internally across 8 cores."""
    X = np.asarray(X, dtype=np.float32)
    # [S,B,D] -> XT [D, NTOK] with b-major tokens (tok = b*S + s)
    xt = np.ascontiguousarray(X.transpose(2, 1, 0).reshape(D, NTOK))
    in_maps = []
    for h in range(N_CORES):
        wqk = np.ascontiguousarray(
            np.concatenate([WQ[h], WK[h]], axis=1), dtype=np.float32)
        bqk = np.ascontiguousarray(
            np.concatenate([bQ[h], bK[h]])[:, None], dtype=np.float32)
        in_maps.append({
            "xt": xt,
            "wqk": wqk,
            "bqk": bqk,
            "wv": np.ascontiguousarray(WV[h], dtype=np.float32),
            "bv": np.ascontiguousarray(bV[h][None, :], dtype=np.float32),
            "wo": np.ascontiguousarray(WO, dtype=np.float32),
            "bo": np.ascontiguousarray(bO[None, :], dtype=np.float32),
        })
    nc = _get_nc()
    res = run_bass_kernel_spmd(
        nc, in_maps, core_ids=list(range(N_CORES)),
        trace=_trace, **(_trace_kwargs or {}),
    )
    # core c rows: [0:512] = tokens (c//2)*S + (c%2)*512 .. ; [512:1024] same + 1024
    fullb = np.empty((B, S, D), dtype=np.float32)
    for c in range(N_CORES):
        oc = res.results[c]["out"]
        b, off = c // 2, (c % 2) * 512
        fullb[b, off:off + 512] = oc[0:512]
        fullb[b, 1024 + off:1024 + off + 512] = oc[512:1024]
    full = fullb.transpose(1, 0, 2)
    if _trace:
        return np.ascontiguousarray(full), res
    return np.ascontiguousarray(full)


# revision 17
# speedup vs baseline: 1.6322x; 1.0076x over previous
"""Trainium2 Bass kernel for nn_Encoder_78795470012907.

Encoder layer: per-head Q/K/V projections, scores = QK^T/sqrt(dk),
double softmax (over batch axis, then over key axis), Z = pV, concat
heads, output projection. S=2048, B=4, D=512, H=8, dk=dv=64.

Sharding: head-parallel over 8 cores (core h owns head h) for the
attention; AllToAll re-shards by token for the output projection, so
each core emits a disjoint 1024-token slice of the output (host just
concatenates).

Layout notes (per core):
 - tokens are b-major: tok = b*2048 + s.
 - X is fed pre-transposed from host as XT [D, NTOK] (pure layout prep).
 - projections produce Q^T/K^T [dk, tok] stacked in b-pairs so the
   scores matmuls row-pack two batches into the 128x128 PE array.
 - scores are computed transposed ([t, s] tiles); the softmax over the
   key axis t rides the Z matmul via a ones-column appended to V
   (row 64 of the Z psum accumulates sum_t exp).
 - softmax over b: e=exp(s/8) -> D=sum_b e -> r=exp(-ln D) -> p1=e*r.
   ln+exp coexist in one ACT table set (no table thrashing);
   Reciprocal would force a table switch per tile.
"""

from contextlib import ExitStack

import numpy as np

import concourse.bass as bass
import concourse.tile as tile
from concourse import bacc, mybir
from concourse.bass_utils import run_bass_kernel_spmd

S, B, D = 2048, 4, 512
H, DK, DV = 8, 64, 64
N_CORES = 8
NTOK = S * B          # 8192 tokens, b-major
TOKC = NTOK // N_CORES  # 1024 tokens per core for the output slice
SC = 512              # s-chunk (columns of a scores^T tile)
TC = 128              # t-chunk (partitions of a scores^T tile)
N_SC = S // SC        # 4
N_TC = S // TC        # 16

F32 = mybir.dt.float32
F32R = mybir.dt.float32r
BF16 = mybir.dt.bfloat16
AF = mybir.ActivationFunctionType


def build_kernel():
    nc = bacc.Bacc(num_devices=N_CORES)

    xt_d = nc.dram_tensor("xt", [D, NTOK], F32, kind="ExternalInput")
    wqk_d = nc.dram_tensor("wqk", [D, 128], F32, kind="ExternalInput")
    bqk_d = nc.dram_tensor("bqk", [128, 1], F32, kind="ExternalInput")
    wv_d = nc.dram_tensor("wv", [D, DV], F32, kind="ExternalInput")
    bv_d = nc.dram_tensor("bv", [1, DV], F32, kind="ExternalInput")
    wo_d = nc.dram_tensor("wo", [D, D], F32, kind="ExternalInput")
    bo_d = nc.dram_tensor("bo", [1, D], F32, kind="ExternalInput")
    out_d = nc.dram_tensor("out", [TOKC, D], F32, kind="ExternalOutput")

    r32 = mybir.dt.float32r

    with tile.TileContext(nc) as tc, ExitStack() as ctx:
        pp = ctx.enter_context(tc.tile_pool(name="persist", bufs=1))
        dram = ctx.enter_context(tc.tile_pool(name="dram", bufs=1, space="DRAM"))

        # ---- persistent SBUF ----
        # Q^T/K^T in b-pairs: rows 0:64 = batch 2p, rows 64:128 = batch 2p+1
        qt = [pp.tile([128, S], BF16, tag=f"qt{p}", name=f"qt{p}") for p in range(2)]
        kt = [pp.tile([128, S], BF16, tag=f"kt{p}", name=f"kt{p}") for p in range(2)]
        # V-tilde: 64 token-chunks of [128 tok, 65] (col 64 = ones)
        vt = pp.tile([128, 64 * 65], BF16, tag="vt", name="vt")
        # Z^T (unnormalized) + denom row: [65, NTOK]
        zt = pp.tile([65, NTOK], BF16, tag="zt", name="zt")

        # weights
        wqk = [pp.tile([128, 128], r32, tag=f"wqk{i}", name=f"wqk{i}") for i in range(4)]
        wv = [pp.tile([128, DV], r32, tag=f"wv{i}", name=f"wv{i}") for i in range(4)]
        wo = [pp.tile([128, D], r32, tag=f"wo{i}", name=f"wo{i}") for i in range(4)]
        bqk = pp.tile([128, 1], F32, tag="bqk", name="bqk")
        bv = pp.tile([1, DV], r32, tag="bv", name="bv")
        bo = pp.tile([1, D], r32, tag="bo", name="bo")
        ones_row = pp.tile([1, 128], r32, tag="ones_row", name="ones_row")

        for i in range(4):
            nc.sync.dma_start(wqk[i][:], wqk_d[i * 128:(i + 1) * 128, :].bitcast(r32))
            nc.sync.dma_start(wv[i][:], wv_d[i * 128:(i + 1) * 128, :].bitcast(r32))
            nc.sync.dma_start(wo[i][:], wo_d[i * 128:(i + 1) * 128, :].bitcast(r32))
        nc.sync.dma_start(bqk[:], bqk_d[:])
        nc.sync.dma_start(bv[:], bv_d[:].bitcast(r32))
        nc.sync.dma_start(bo[:], bo_d[:].bitcast(r32))
        # memset is ISA-invalid for f32r tiles; stage ones in f32 and copy.
        onesf = pp.tile([128, 128], F32, tag="onesf", name="onesf")
        nc.vector.memset(onesf[:], 1.0)
        nc.vector.tensor_copy(ones_row[:], onesf[0:1, :])
        # ones column (col 64 of each 65-wide group) of V-tilde
        vt_ones = vt[:].rearrange("p (n c) -> p n c", c=65)[:, :, 64:65]
        nc.vector.tensor_copy(vt_ones, onesf[:, 0:64, None])

        # ================= Phase A: projections =================
        with (
            tc.tile_pool(name="xtp", bufs=2) as xp,
            tc.tile_pool(name="psA", bufs=2, space="PSUM") as psA,
        ):
            # b-inner order so the first 4 chunks cover (sc=0, t=0..3) of
            # every batch - lets attention start ~4x earlier
            for ck in [b * 4 + ssub for ssub in range(4) for b in range(4)]:
                b = ck // 4
                pair, row = b // 2, (b % 2) * 64
                xtile = [xp.tile([128, 512], r32, tag=f"xt{i}", name=f"xtile{i}") for i in range(4)]
                for i in range(4):
                    nc.sync.dma_start(
                        xtile[i][:],
                        xt_d[i * 128:(i + 1) * 128, ck * 512:(ck + 1) * 512].bitcast(r32),
                    )
                # Q^T | K^T (stacked 64+64) for this token chunk
                pqk = psA.tile([128, 512], F32, tag="pqk", name="pqk")
                for i in range(4):
                    nc.tensor.matmul(pqk[:], wqk[i][:], xtile[i][:],
                                     start=(i == 0), stop=(i == 3))
                scol = (ck % 4) * 512
                nc.scalar.activation(qt[pair][row:row + 64, scol:scol + 512],
                                     pqk[0:64, :], AF.Identity, bias=bqk[0:64, :])
                nc.scalar.activation(kt[pair][row:row + 64, scol:scol + 512],
                                     pqk[64:128, :], AF.Identity, bias=bqk[64:128, :])
                # V (natural layout) per 128-token subchunk, + ones-row bias fold
                for sub in range(4):
                    pv = psA.tile([128, DV], F32, tag="pv", name="pv")
                    for i in range(4):
                        nc.tensor.matmul(pv[:], xtile[i][:, sub * 128:(sub + 1) * 128],
                                         wv[i][:], start=(i == 0), stop=False)
                    nc.tensor.matmul(pv[:], ones_row[:], bv[:], start=False, stop=True)
                    tci = ck * 4 + sub  # global token-chunk index (b-major)
                    nc.vector.tensor_copy(vt[:, tci * 65:tci * 65 + 64], pv[:])

        # ================= Phase B: attention =================
        with (
            tc.tile_pool(name="wb", bufs=2) as wb,
            tc.tile_pool(name="psB", bufs=1, space="PSUM") as psB,
        ):
            # Software-pipelined over 64 global blocks g = sc*16 + t.
            # Per iteration: scores(g)+exp1(g) are emitted BEFORE the
            # DVE chain of g and exp2(g-1), so the ACT queue interleaves
            # exp1(g+1) ahead of exp2(g) and blocks overlap.
            NB = N_SC * N_TC
            pipe = {}  # g -> (e, p1) tiles

            def softmax_b(g):
                """scores(g) -> e(g) -> p1(g) tiles (no exp2 yet)."""
                sc, t = g // N_TC, g % N_TC
                scp = psB.tile([128, 4 * SC], F32, tag="scp", name="scp")
                for b in range(4):
                    pair, row = b // 2, (b % 2) * 64
                    nc.tensor.matmul(
                        scp[:, b * SC:(b + 1) * SC],
                        kt[pair][row:row + 64, t * TC:(t + 1) * TC],
                        qt[pair][row:row + 64, sc * SC:(sc + 1) * SC],
                        start=True, stop=True,
                    )
                # e = exp(scores/8) for all 4 b
                e = wb.tile([128, 4 * SC], BF16, tag="e", name="e", bufs=3)
                nc.scalar.activation(e[:], scp[:], AF.Exp, scale=0.125)
                # D = sum_b e ; r = 1/D (custom-DVE fast reciprocal keeps
                # ACT on the single exp table set - no table thrashing)
                t01 = wb.tile([128, 2 * SC], BF16, tag="t01", name="t01", bufs=2)
                nc.vector.tensor_add(t01[:], e[:, 0:2 * SC], e[:, 2 * SC:4 * SC])
                dd = wb.tile([128, SC], BF16, tag="dd", name="dd", bufs=2)
                nc.vector.tensor_add(dd[:], t01[:, 0:SC], t01[:, SC:2 * SC])
                ddf = wb.tile([128, SC], F32, tag="ddf", name="ddf", bufs=2)
                nc.vector.tensor_copy(ddf[:], dd[:])
                rf = wb.tile([128, SC], F32, tag="rf", name="rf", bufs=2)
                nc.vector.reciprocal_approx_fast(rf[:], ddf[:])
                rr = wb.tile([128, SC], BF16, tag="rr", name="rr", bufs=2)
                nc.vector.tensor_copy(rr[:], rf[:])
                # p1 = e * r, one TT with r broadcast along the 4-b free dim
                p1 = wb.tile([128, 4 * SC], BF16, tag="p1", name="p1", bufs=3)
                nc.vector.tensor_mul(
                    p1[:].rearrange("p (b s) -> p b s", b=4),
                    e[:].rearrange("p (b s) -> p b s", b=4),
                    rr[:, None, :].broadcast_to([128, 4, SC]),
                )
                pipe[g] = p1

            def exp2_and_z(g, zacc):
                """exp2(g) + Z accumulation (ones-col -> sum_t in row 64)."""
                t = g % N_TC
                p1 = pipe.pop(g)
                q = wb.tile([128, 4 * SC], BF16, tag="q", name="q", bufs=3)
                nc.scalar.activation(q[:], p1[:], AF.Exp)
                for b in range(4):
                    tci = b * 16 + t
                    nc.tensor.matmul(
                        zacc[:, b * SC:(b + 1) * SC],
                        vt[:, tci * 65:(tci + 1) * 65],
                        q[:, b * SC:(b + 1) * SC],
                        start=(t == 0), stop=(t == N_TC - 1),
                    )

            a2a_in_h = [dram.tile([N_CORES * 65, 512], BF16, tag=f"a2a_in{q}",
                                  name=f"a2a_in{q}") for q in range(2)]
            a2a_out_h = [dram.tile([N_CORES * 65, 512], BF16, tag=f"a2a_out{q}",
                                   name=f"a2a_out{q}") for q in range(2)]

            def emit_a2a(q):
                # chunk r = my head's Z^T cols for core r's half-q tokens:
                # tok = (r//2)*S + q*1024 + (r%2)*512 ... +512
                for r in range(N_CORES):
                    col = (r // 2) * S + q * 1024 + (r % 2) * 512
                    nc.sync.dma_start(a2a_in_h[q][r * 65:(r + 1) * 65, :],
                                      zt[:, col:col + 512])
                nc.gpsimd.collective_compute(
                    "AllToAll",
                    mybir.AluOpType.bypass,
                    replica_groups=[list(range(N_CORES))],
                    ins=[a2a_in_h[q][:].opt()],
                    outs=[a2a_out_h[q][:].opt()],
                )

            zaccs = {}
            for g in range(NB + 2):
                if g < NB:
                    if g % N_TC == 0:
                        zaccs[g // N_TC] = psB.tile([65, 4 * SC], F32,
                                                    tag="zacc", name="zacc")
                    softmax_b(g)
                if g >= 2:
                    gz = g - 2
                    za = zaccs[gz // N_TC]
                    exp2_and_z(gz, za)
                    if gz % N_TC == N_TC - 1:
                        # evacuate Z^T (+denominator row) to bf16
                        sc_done = gz // N_TC
                        for b in range(4):
                            col = b * S + sc_done * SC
                            nc.vector.tensor_copy(zt[:, col:col + SC],
                                                  za[:, b * SC:(b + 1) * SC])
                        if sc_done == 1:
                            emit_a2a(0)  # overlaps remaining attention
                        elif sc_done == 3:
                            emit_a2a(1)

        # ================= Phase C: output projection =================
        with (
            tc.tile_pool(name="wc", bufs=1) as wc,
            tc.tile_pool(name="oc", bufs=2) as oc,
            tc.tile_pool(name="psC", bufs=2, space="PSUM") as psC,
        ):
            HT = 512  # tokens per half
            for q in range(2):
                zc = wc.tile([65, N_CORES * HT], BF16, tag="zc", name="zc", bufs=2)
                for j in range(N_CORES):
                    nc.sync.dma_start(zc[:, j * HT:(j + 1) * HT],
                                      a2a_out_h[q][j * 65:(j + 1) * 65, :])
                # r2 = exp(-ln(denom)) per (head, token); bcast to 64 rows
                lden = wc.tile([1, N_CORES * HT], F32, tag="lden", name="lden", bufs=2)
                nc.scalar.activation(lden[:], zc[64:65, :], AF.Ln)
                rden = wc.tile([1, N_CORES * HT], BF16, tag="rden", name="rden", bufs=2)
                nc.scalar.activation(rden[:], lden[:], AF.Exp, scale=-1.0)
                rb = wc.tile([64, N_CORES * HT], BF16, tag="rb", name="rb", bufs=2)
                nc.gpsimd.partition_broadcast(rb[:], rden[:])
                # normalized Zc^T in hd-major pairs: tile i = heads 2i, 2i+1
                zcn = [wc.tile([128, HT], r32, tag=f"zcn{i}", name=f"zcn{i}", bufs=2)
                       for i in range(4)]
                for j in range(N_CORES):
                    nc.vector.tensor_mul(
                        zcn[j // 2][(j % 2) * 64:(j % 2) * 64 + 64, :],
                        zc[0:64, j * HT:(j + 1) * HT],
                        rb[:, j * HT:(j + 1) * HT],
                    )
                for m in range(HT // 128):
                    po = psC.tile([128, D], F32, tag="po", name="po")
                    for i in range(4):
                        nc.tensor.matmul(po[:], zcn[i][:, m * 128:(m + 1) * 128],
                                         wo[i][:], start=(i == 0), stop=False)
                    nc.tensor.matmul(po[:], ones_row[:], bo[:], start=False,
                                     stop=True)
                    ot = oc.tile([128, D], F32, tag="ot", name="ot")
                    nc.vector.tensor_copy(ot[:], po[:])
                    row = q * HT + m * 128
                    nc.sync.dma_start(out_d[row:row + 128, :], ot[:])

    nc.compile()
    return nc


_NC_CACHE = None


def _get_nc():
    global _NC_CACHE
    if _NC_CACHE is None:
        _NC_CACHE = build_kernel()
    return _NC_CACHE


def kernel(X, WQ, bQ, WK, bK, WV, bV, WO, bO, _trace=False, _trace_kwargs=None):
    """Full inputs in, full output out. Shards internally across 8 cores."""
    X = np.asarray(X, dtype=np.float32)
    # [S,B,D] -> XT [D, NTOK] with b-major tokens (tok = b*S + s)
    xt = np.ascontiguousarray(X.transpose(2, 1, 0).reshape(D, NTOK))
    in_maps = []
    for h in range(N_CORES):
        wqk = np.ascontiguousarray(
            np.concatenate([WQ[h], WK[h]], axis=1), dtype=np.float32)
        bqk = np.ascontiguousarray(
            np.concatenate([bQ[h], bK[h]])[:, None], dtype=np.float32)
        in_maps.append({
            "xt": xt,
            "wqk": wqk,
            "bqk": bqk,
            "wv": np.ascontiguousarray(WV[h], dtype=np.float32),
            "bv": np.ascontiguousarray(bV[h][None, :], dtype=np.float32),
            "wo": np.ascontiguousarray(WO, dtype=np.float32),
            "bo": np.ascontiguousarray(bO[None, :], dtype=np.float32),
        })
    nc = _get_nc()
    res = run_bass_kernel_spmd(
        nc, in_maps, core_ids=list(range(N_CORES)),
        trace=_trace, **(_trace_kwargs or {}),
    )
    # core c rows: [0:512] = tokens (c//2)*S + (c%2)*512 .. ; [512:1024] same + 1024
    fullb = np.empty((B, S, D), dtype=np.float32)
    for c in range(N_CORES):
        oc = res.results[c]["out"]
        b, off = c // 2, (c % 2) * 512
        fullb[b, off:off + 512] = oc[0:512]
        fullb[b, 1024 + off:1024 + off + 512] = oc[512:1024]
    full = fullb.transpose(1, 0, 2)
    if _trace:
        return np.ascontiguousarray(full), res
    return np.ascontiguousarray(full)


# revision 19
# speedup vs baseline: 1.8136x; 1.1111x over previous
"""Trainium2 Bass kernel for nn_Encoder_78795470012907.

Encoder layer: per-head Q/K/V projections, scores = QK^T/sqrt(dk),
double softmax (over batch axis, then over key axis), Z = pV, concat
heads, output projection. S=2048, B=4, D=512, H=8, dk=dv=64.

Sharding: head-parallel over 8 cores (core h owns head h) for the
attention; AllToAll re-shards by token for the output projection, so
each core emits a disjoint 1024-token slice of the output (host just
concatenates).

Layout notes (per core):
 - tokens are b-major: tok = b*2048 + s.
 - X is fed pre-transposed from host as XT [D, NTOK] (pure layout prep).
 - projections produce Q^T/K^T [dk, tok] stacked in b-pairs so the
   scores matmuls row-pack two batches into the 128x128 PE array.
 - scores are computed transposed ([t, s] tiles); the softmax over the
   key axis t rides the Z matmul via a ones-column appended to V
   (row 64 of the Z psum accumulates sum_t exp).
 - softmax over b: e=exp(s/8) -> D=sum_b e -> r=1/D -> p1=e*r, with
   1/D on the custom-DVE fast reciprocal so ACT stays on one exp table
   set (a Reciprocal/Ln activation would thrash ACT_TABLE_LOAD per tile).
 - phase B is software-pipelined two blocks deep so exp1(g+2) precedes
   exp2(g) in the ACT queue; the AllToAll is split in two halves, the
   first overlapping the second half of the attention loop.
"""

from contextlib import ExitStack

import numpy as np

import concourse.bass as bass
import concourse.tile as tile
from concourse import bacc, mybir
from concourse.bass_utils import run_bass_kernel_spmd

S, B, D = 2048, 4, 512
H, DK, DV = 8, 64, 64
N_CORES = 8
NTOK = S * B          # 8192 tokens, b-major
TOKC = NTOK // N_CORES  # 1024 tokens per core for the output slice
SC = 512              # s-chunk (columns of a scores^T tile)
TC = 128              # t-chunk (partitions of a scores^T tile)
N_SC = S // SC        # 4
N_TC = S // TC        # 16

F32 = mybir.dt.float32
F32R = mybir.dt.float32r
BF16 = mybir.dt.bfloat16
AF = mybir.ActivationFunctionType


def build_kernel():
    nc = bacc.Bacc(num_devices=N_CORES)

    xt_d = nc.dram_tensor("xt", [D, NTOK], F32, kind="ExternalInput")
    wqk_d = nc.dram_tensor("wqk", [D, 128], F32, kind="ExternalInput")
    bqk_d = nc.dram_tensor("bqk", [128, 1], F32, kind="ExternalInput")
    wv_d = nc.dram_tensor("wv", [D, DV], F32, kind="ExternalInput")
    bv_d = nc.dram_tensor("bv", [1, DV], F32, kind="ExternalInput")
    wo_d = nc.dram_tensor("wo", [D, D], F32, kind="ExternalInput")
    bo_d = nc.dram_tensor("bo", [1, D], F32, kind="ExternalInput")
    out_d = nc.dram_tensor("out", [TOKC, D], F32, kind="ExternalOutput")

    r32 = mybir.dt.float32r

    with tile.TileContext(nc) as tc, ExitStack() as ctx:
        pp = ctx.enter_context(tc.tile_pool(name="persist", bufs=1))
        dram = ctx.enter_context(tc.tile_pool(name="dram", bufs=1, space="DRAM"))

        # ---- persistent SBUF ----
        # Q^T/K^T in b-pairs: rows 0:64 = batch 2p, rows 64:128 = batch 2p+1
        qt = [pp.tile([128, S], BF16, tag=f"qt{p}", name=f"qt{p}") for p in range(2)]
        kt = [pp.tile([128, S], BF16, tag=f"kt{p}", name=f"kt{p}") for p in range(2)]
        # V-tilde: 64 token-chunks of [128 tok, 65] (col 64 = ones)
        vt = pp.tile([128, 64 * 65], BF16, tag="vt", name="vt")
        # Z^T (unnormalized) + denom row: [65, NTOK]
        zt = pp.tile([65, NTOK], BF16, tag="zt", name="zt")

        # weights
        wqk = [pp.tile([128, 128], r32, tag=f"wqk{i}", name=f"wqk{i}") for i in range(4)]
        wv = [pp.tile([128, DV], r32, tag=f"wv{i}", name=f"wv{i}") for i in range(4)]
        wo = [pp.tile([128, D], r32, tag=f"wo{i}", name=f"wo{i}") for i in range(4)]
        bqk = pp.tile([128, 1], F32, tag="bqk", name="bqk")
        bv = pp.tile([1, DV], r32, tag="bv", name="bv")
        bo = pp.tile([1, D], r32, tag="bo", name="bo")
        ones_row = pp.tile([1, 128], r32, tag="ones_row", name="ones_row")

        for i in range(4):
            nc.sync.dma_start(wqk[i][:], wqk_d[i * 128:(i + 1) * 128, :].bitcast(r32))
            nc.sync.dma_start(wv[i][:], wv_d[i * 128:(i + 1) * 128, :].bitcast(r32))
            nc.sync.dma_start(wo[i][:], wo_d[i * 128:(i + 1) * 128, :].bitcast(r32))
        nc.sync.dma_start(bqk[:], bqk_d[:])
        nc.sync.dma_start(bv[:], bv_d[:].bitcast(r32))
        nc.sync.dma_start(bo[:], bo_d[:].bitcast(r32))
        # memset is ISA-invalid for f32r tiles; stage ones in f32 and copy.
        onesf = pp.tile([128, 128], F32, tag="onesf", name="onesf")
        nc.vector.memset(onesf[:], 1.0)
        nc.vector.tensor_copy(ones_row[:], onesf[0:1, :])
        # ones column (col 64 of each 65-wide group) of V-tilde
        vt_ones = vt[:].rearrange("p (n c) -> p n c", c=65)[:, :, 64:65]
        nc.vector.tensor_copy(vt_ones, onesf[:, 0:64, None])
        # bf16 V-projection operands: bf16 matmuls get fast weight load and
        # 1 cyc/row at N=64, vs ~480ns/mm for f32r (LDW 271ns + 4 cyc/row)
        wv_bf = [pp.tile([128, DV], BF16, tag=f"wvb{i}", name=f"wvb{i}")
                 for i in range(4)]
        for i in range(4):
            nc.vector.tensor_copy(wv_bf[i][:], wv[i][:])
        ones_bf = pp.tile([1, 128], BF16, tag="ones_bf", name="ones_bf")
        nc.vector.tensor_copy(ones_bf[:], onesf[0:1, :])
        bv_bf = pp.tile([1, DV], BF16, tag="bv_bf", name="bv_bf")
        nc.vector.tensor_copy(bv_bf[:], bv[:])

        # ================= Phase A: projections =================
        with (
            tc.tile_pool(name="xtp", bufs=2) as xp,
            tc.tile_pool(name="psA", bufs=2, space="PSUM") as psA,
        ):
            # b-inner order so the first 4 chunks cover (sc=0, t=0..3) of
            # every batch - lets attention start ~4x earlier
            for ck in [b * 4 + ssub for ssub in range(4) for b in range(4)]:
                b = ck // 4
                pair, row = b // 2, (b % 2) * 64
                xtile = [xp.tile([128, 512], r32, tag=f"xt{i}", name=f"xtile{i}") for i in range(4)]
                for i in range(4):
                    nc.sync.dma_start(
                        xtile[i][:],
                        xt_d[i * 128:(i + 1) * 128, ck * 512:(ck + 1) * 512].bitcast(r32),
                    )
                # Q^T | K^T (stacked 64+64) for this token chunk
                pqk = psA.tile([128, 512], F32, tag="pqk", name="pqk")
                for i in range(4):
                    nc.tensor.matmul(pqk[:], wqk[i][:], xtile[i][:],
                                     start=(i == 0), stop=(i == 3))
                scol = (ck % 4) * 512
                nc.scalar.activation(qt[pair][row:row + 64, scol:scol + 512],
                                     pqk[0:64, :], AF.Identity, bias=bqk[0:64, :])
                nc.scalar.activation(kt[pair][row:row + 64, scol:scol + 512],
                                     pqk[64:128, :], AF.Identity, bias=bqk[64:128, :])
                # V (natural layout) per 128-token subchunk, + ones-row bias fold
                xbf = [xp.tile([128, 512], BF16, tag=f"xbf{i}", name=f"xbf{i}")
                       for i in range(4)]
                for i in range(4):
                    nc.vector.tensor_copy(xbf[i][:], xtile[i][:])
                for sub in range(4):
                    pv = psA.tile([128, DV], F32, tag="pv", name="pv")
                    for i in range(4):
                        nc.tensor.matmul(pv[:], xbf[i][:, sub * 128:(sub + 1) * 128],
                                         wv_bf[i][:], start=(i == 0), stop=False)
                    nc.tensor.matmul(pv[:], ones_bf[:], bv_bf[:], start=False, stop=True)
                    tci = ck * 4 + sub  # global token-chunk index (b-major)
                    nc.vector.tensor_copy(vt[:, tci * 65:tci * 65 + 64], pv[:])

        # ================= Phase B: attention =================
        with (
            tc.tile_pool(name="wb", bufs=2) as wb,
            tc.tile_pool(name="psB", bufs=1, space="PSUM") as psB,
        ):
            # Software-pipelined over 64 global blocks g = sc*16 + t.
            # Per iteration: scores(g)+exp1(g) are emitted BEFORE the
            # DVE chain of g and exp2(g-1), so the ACT queue interleaves
            # exp1(g+1) ahead of exp2(g) and blocks overlap.
            NB = N_SC * N_TC
            pipe = {}  # g -> (e, p1) tiles

            def softmax_b(g):
                """scores(g) -> e(g) -> p1(g) tiles (no exp2 yet)."""
                sc, t = g // N_TC, g % N_TC
                scp = psB.tile([128, 4 * SC], F32, tag="scp", name="scp")
                for b in range(4):
                    pair, row = b // 2, (b % 2) * 64
                    nc.tensor.matmul(
                        scp[:, b * SC:(b + 1) * SC],
                        kt[pair][row:row + 64, t * TC:(t + 1) * TC],
                        qt[pair][row:row + 64, sc * SC:(sc + 1) * SC],
                        start=True, stop=True,
                    )
                # e = exp(scores/8) for all 4 b
                e = wb.tile([128, 4 * SC], BF16, tag="e", name="e", bufs=3)
                nc.scalar.activation(e[:], scp[:], AF.Exp, scale=0.125)
                # D = sum_b e ; r = 1/D (custom-DVE fast reciprocal keeps
                # ACT on the single exp table set - no table thrashing)
                t01 = wb.tile([128, 2 * SC], BF16, tag="t01", name="t01", bufs=2)
                nc.vector.tensor_add(t01[:], e[:, 0:2 * SC], e[:, 2 * SC:4 * SC])
                dd = wb.tile([128, SC], BF16, tag="dd", name="dd", bufs=2)
                nc.vector.tensor_add(dd[:], t01[:, 0:SC], t01[:, SC:2 * SC])
                ddf = wb.tile([128, SC], F32, tag="ddf", name="ddf", bufs=2)
                nc.vector.tensor_copy(ddf[:], dd[:])
                rf = wb.tile([128, SC], F32, tag="rf", name="rf", bufs=2)
                nc.vector.reciprocal_approx_fast(rf[:], ddf[:])
                rr = wb.tile([128, SC], BF16, tag="rr", name="rr", bufs=2)
                nc.vector.tensor_copy(rr[:], rf[:])
                # p1 = e * r, one TT with r broadcast along the 4-b free dim
                p1 = wb.tile([128, 4 * SC], BF16, tag="p1", name="p1", bufs=3)
                nc.vector.tensor_mul(
                    p1[:].rearrange("p (b s) -> p b s", b=4),
                    e[:].rearrange("p (b s) -> p b s", b=4),
                    rr[:, None, :].broadcast_to([128, 4, SC]),
                )
                pipe[g] = p1

            def exp2_and_z(g, zacc):
                """exp2(g) + Z accumulation (ones-col -> sum_t in row 64)."""
                t = g % N_TC
                p1 = pipe.pop(g)
                q = wb.tile([128, 4 * SC], BF16, tag="q", name="q", bufs=3)
                nc.scalar.activation(q[:], p1[:], AF.Exp)
                for b in range(4):
                    tci = b * 16 + t
                    nc.tensor.matmul(
                        zacc[:, b * SC:(b + 1) * SC],
                        vt[:, tci * 65:(tci + 1) * 65],
                        q[:, b * SC:(b + 1) * SC],
                        start=(t == 0), stop=(t == N_TC - 1),
                    )

            a2a_in_h = [dram.tile([N_CORES * 65, 512], BF16, tag=f"a2a_in{q}",
                                  name=f"a2a_in{q}") for q in range(2)]
            a2a_out_h = [dram.tile([N_CORES * 65, 512], BF16, tag=f"a2a_out{q}",
                                   name=f"a2a_out{q}") for q in range(2)]

            def emit_a2a(q):
                # chunk r = my head's Z^T cols for core r's half-q tokens:
                # tok = (r//2)*S + q*1024 + (r%2)*512 ... +512
                for r in range(N_CORES):
                    col = (r // 2) * S + q * 1024 + (r % 2) * 512
                    nc.sync.dma_start(a2a_in_h[q][r * 65:(r + 1) * 65, :],
                                      zt[:, col:col + 512])
                nc.gpsimd.collective_compute(
                    "AllToAll",
                    mybir.AluOpType.bypass,
                    replica_groups=[list(range(N_CORES))],
                    ins=[a2a_in_h[q][:].opt()],
                    outs=[a2a_out_h[q][:].opt()],
                )

            zaccs = {}
            for g in range(NB + 2):
                if g < NB:
                    if g % N_TC == 0:
                        zaccs[g // N_TC] = psB.tile([65, 4 * SC], F32,
                                                    tag="zacc", name="zacc")
                    softmax_b(g)
                if g >= 2:
                    gz = g - 2
                    za = zaccs[gz // N_TC]
                    exp2_and_z(gz, za)
                    if gz % N_TC == N_TC - 1:
                        # evacuate Z^T (+denominator row) to bf16
                        sc_done = gz // N_TC
                        for b in range(4):
                            col = b * S + sc_done * SC
                            nc.vector.tensor_copy(zt[:, col:col + SC],
                                                  za[:, b * SC:(b + 1) * SC])
                        if sc_done == 1:
                            emit_a2a(0)  # overlaps remaining attention
                        elif sc_done == 3:
                            emit_a2a(1)

        # ================= Phase C: output projection =================
        with (
            tc.tile_pool(name="wc", bufs=1) as wc,
            tc.tile_pool(name="oc", bufs=2) as oc,
            tc.tile_pool(name="psC", bufs=2, space="PSUM") as psC,
        ):
            HT = 512  # tokens per half
            for q in range(2):
                zc = wc.tile([65, N_CORES * HT], BF16, tag="zc", name="zc", bufs=2)
                for j in range(N_CORES):
                    nc.sync.dma_start(zc[:, j * HT:(j + 1) * HT],
                                      a2a_out_h[q][j * 65:(j + 1) * 65, :])
                # r2 = exp(-ln(denom)) per (head, token); bcast to 64 rows
                lden = wc.tile([1, N_CORES * HT], F32, tag="lden", name="lden", bufs=2)
                nc.scalar.activation(lden[:], zc[64:65, :], AF.Ln)
                rden = wc.tile([1, N_CORES * HT], BF16, tag="rden", name="rden", bufs=2)
                nc.scalar.activation(rden[:], lden[:], AF.Exp, scale=-1.0)
                rb = wc.tile([64, N_CORES * HT], BF16, tag="rb", name="rb", bufs=2)
                nc.gpsimd.partition_broadcast(rb[:], rden[:])
                # normalized Zc^T in hd-major pairs: tile i = heads 2i, 2i+1
                zcn = [wc.tile([128, HT], r32, tag=f"zcn{i}", name=f"zcn{i}", bufs=2)
                       for i in range(4)]
                for j in range(N_CORES):
                    nc.vector.tensor_mul(
                        zcn[j // 2][(j % 2) * 64:(j % 2) * 64 + 64, :],
                        zc[0:64, j * HT:(j + 1) * HT],
                        rb[:, j * HT:(j + 1) * HT],
                    )
                for m in range(HT // 128):
                    po = psC.tile([128, D], F32, tag="po", name="po")
                    for i in range(4):
                        nc.tensor.matmul(po[:], zcn[i][:, m * 128:(m + 1) * 128],
                                         wo[i][:], start=(i == 0), stop=False)
                    nc.tensor.matmul(po[:], ones_row[:], bo[:], start=False,
                                     stop=True)
                    ot = oc.tile([128, D], F32, tag="ot", name="ot")
                    nc.vector.tensor_copy(ot[:], po[:])
                    row = q * HT + m * 128
                    nc.sync.dma_start(out_d[row:row + 128, :], ot[:])

    nc.compile()
    return nc


_NC_CACHE = None


def _get_nc():
    global _NC_CACHE
    if _NC_CACHE is None:
        _NC_CACHE = build_kernel()
    return _NC_CACHE


def kernel(X, WQ, bQ, WK, bK, WV, bV, WO, bO, _trace=False, _trace_kwargs=None):
    """Full inputs in, full output out. Shards internally across 8 cores."""
    X = np.asarray(X, dtype=np.float32)
    WQ, bQ = np.asarray(WQ, np.float32), np.asarray(bQ, np.float32)
    WK, bK = np.asarray(WK, np.float32), np.asarray(bK, np.float32)
    WV, bV = np.asarray(WV, np.float32), np.asarray(bV, np.float32)
    WO, bO = np.asarray(WO, np.float32), np.asarray(bO, np.float32)
    # [S,B,D] -> XT [D, NTOK] with b-major tokens (tok = b*S + s)
    xt = np.ascontiguousarray(X.transpose(2, 1, 0).reshape(D, NTOK))
    in_maps = []
    for h in range(N_CORES):
        wqk = np.ascontiguousarray(
            np.concatenate([WQ[h], WK[h]], axis=1), dtype=np.float32)
        bqk = np.ascontiguousarray(
            np.concatenate([bQ[h], bK[h]])[:, None], dtype=np.float32)
        in_maps.append({
            "xt": xt,
            "wqk": wqk,
            "bqk": bqk,
            "wv": np.ascontiguousarray(WV[h], dtype=np.float32),
            "bv": np.ascontiguousarray(bV[h][None, :], dtype=np.float32),
            "wo": np.ascontiguousarray(WO, dtype=np.float32),
            "bo": np.ascontiguousarray(bO[None, :], dtype=np.float32),
        })
    nc = _get_nc()
    res = run_bass_kernel_spmd(
        nc, in_maps, core_ids=list(range(N_CORES)),
        trace=_trace, **(_trace_kwargs or {}),
    )
    # core c rows: [0:512] = tokens (c//2)*S + (c%2)*512 .. ; [512:1024] same + 1024
    fullb = np.empty((B, S, D), dtype=np.float32)
    for c in range(N_CORES):
        oc = res.results[c]["out"]
        b, off = c // 2, (c % 2) * 512
        fullb[b, off:off + 512] = oc[0:512]
        fullb[b, 1024 + off:1024 + off + 512] = oc[512:1024]
    full = fullb.transpose(1, 0, 2)
    if _trace:
        return np.ascontiguousarray(full), res
    return np.ascontiguousarray(full)
